# revision 25
# baseline (speedup 1.0000x reference)
"""Multi-Head Latent Attention (MLA) prefill kernel for 8 Trainium2 NeuronCores.

Problem shapes: B=2, S=2048, DIM=2048, H=16 heads, HEAD_DIM=128, LATENT=512.

Strategy (tensor-parallel over heads + data-parallel phase A):
  Phase A (token-DP): each core computes the latent down-projections
    c_kvT, c_qT and the rope projections k_rT, q_rT (pe-scaled) for its
    512-token shard, in transposed [feature, token] layout, fp16.
  AllGather (x2, overlapped): shards are exchanged so every core holds the
    full-sequence latents/ropes (~3.4MB/rank fp16 total).
  Phase B (head-TP): each core serves 2 of the 16 heads: up-projects
    k_c/q_c (transposed) and v (natural, with a ones-column appended so the
    softmax denominator falls out of the ctx matmul for free), then runs
    attention (scores^T tiles -> exp on ACT -> ctx accumulate in PSUM ->
    per-row normalize -> PE transpose -> W_O row-slice partial product).
  Host: sums the 8 partial outputs and adds b_O.

All matmuls run with fp16 operands and fp32 PSUM accumulation
(measured end-to-end rel. error ~3e-4 in numpy simulation).
"""
import math

import numpy as np

import concourse.bacc as bacc
import concourse.mybir as mybir
import concourse.tile as tile
from concourse import bass_utils
from concourse.masks import make_identity

# Problem constants (hardcoded per harness contract).
B, S, DIM, H, HD, LAT = 2, 2048, 2048, 16, 128, 512
N_CORES = 8
HPC = H // N_CORES          # heads per core = 2
TOK = B * S                 # 4096 tokens
TPC = TOK // N_CORES        # 512 tokens per core (phase A shard)
EC = DIM // 128             # 16 embedding chunks
LC = LAT // 128             # 4 latent chunks
RB = N_CORES                # rank blocks of TPC tokens
SBLK = 512                  # query block (phase B)
NSB = S // SBLK             # 4 s-blocks per batch
TC_B = S // 128             # 16 key chunks per batch
F16 = mybir.dt.float16
F32 = mybir.dt.float32
SCALE = 1.0 / math.sqrt(HD)

_CACHE = {}


OPTS = dict(psS_bufs=2, psC_bufs=1, psT_bufs=1, psO_bufs=2, est_bufs=2)


def _build(use_cc=True, n_devices=N_CORES, **opts):
    o = dict(OPTS)
    o.update(opts)
    nc = bacc.Bacc("TRN2", target_bir_lowering=False, debug=False,
                   num_devices=n_devices)

    # ---- per-core external inputs ----
    xT = nc.dram_tensor("xT", [DIM, TPC], F16, kind="ExternalInput")
    wdkv = nc.dram_tensor("wdkv", [DIM, LAT], F16, kind="ExternalInput")
    wdq = nc.dram_tensor("wdq", [DIM, LAT], F16, kind="ExternalInput")
    wkr = nc.dram_tensor("wkr", [DIM, HD], F16, kind="ExternalInput")
    wqr = nc.dram_tensor("wqr", [DIM, HD], F16, kind="ExternalInput")
    bdkv = nc.dram_tensor("bdkv", [LAT], F32, kind="ExternalInput")
    bdq = nc.dram_tensor("bdq", [LAT], F32, kind="ExternalInput")
    bkr = nc.dram_tensor("bkr", [HD], F32, kind="ExternalInput")
    bqr = nc.dram_tensor("bqr", [HD], F32, kind="ExternalInput")
    pet = nc.dram_tensor("pet", [HD, TPC], F32, kind="ExternalInput")
    wuk = nc.dram_tensor("wuk", [LAT, HPC * HD], F16, kind="ExternalInput")
    wuv = nc.dram_tensor("wuv", [LAT, HPC * HD], F16, kind="ExternalInput")
    wuq = nc.dram_tensor("wuq", [LAT, HPC * HD], F16, kind="ExternalInput")
    buk = nc.dram_tensor("buk", [HPC * HD], F32, kind="ExternalInput")
    buv = nc.dram_tensor("buv", [HPC * HD], F32, kind="ExternalInput")
    buq = nc.dram_tensor("buq", [HPC * HD], F32, kind="ExternalInput")
    wo = nc.dram_tensor("wo", [HPC * HD, DIM], F16, kind="ExternalInput")

    out_part = nc.dram_tensor("out_part", [TOK, DIM], F16,
                              kind="ExternalOutput")

    CKV_N = LAT * TPC            # 262144 elems per rank shard
    AUX_N = LAT * TPC + 2 * HD * TPC  # c_qT + k_rT + q_rT = 393216
    KR_OFF = LAT * TPC
    QR_OFF = KR_OFF + HD * TPC

    with tile.TileContext(nc) as tc:
        with tc.tile_pool(name="dram", bufs=1, space="DRAM") as dram:
            bin1 = dram.tile([1, CKV_N], F16)
            bout1 = dram.tile([RB, CKV_N], F16, addr_space="Shared")
            bin2 = dram.tile([1, AUX_N], F16)
            bout2 = dram.tile([RB, AUX_N], F16, addr_space="Shared")

            # ================= Phase A (token shard, transposed outputs) ===
            with tc.tile_pool(name="pA", bufs=1) as pA, \
                 tc.tile_pool(name="psA", bufs=2, space="PSUM") as psA:
                x_sb = pA.tile([128, EC, TPC], F16)
                xT_r = xT.ap().rearrange("(n p) f -> p n f", p=128)
                nc.sync.dma_start(out=x_sb[:, 0:4], in_=xT_r[:, 0:4])
                nc.sync.dma_start(out=x_sb[:, 4:EC], in_=xT_r[:, 4:EC])
                wdkv_sb = pA.tile([128, EC, LAT], F16)
                wdkv_r = wdkv.ap().rearrange("(n p) f -> p n f", p=128)
                nc.sync.dma_start(out=wdkv_sb[:, 0:4], in_=wdkv_r[:, 0:4])
                nc.sync.dma_start(out=wdkv_sb[:, 4:EC], in_=wdkv_r[:, 4:EC])
                wdq_sb = pA.tile([128, EC, LAT], F16)
                nc.sync.dma_start(
                    out=wdq_sb,
                    in_=wdq.ap().rearrange("(n p) f -> p n f", p=128))
                wkr_sb = pA.tile([128, EC, HD], F16)
                nc.sync.dma_start(
                    out=wkr_sb,
                    in_=wkr.ap().rearrange("(n p) f -> p n f", p=128))
                wqr_sb = pA.tile([128, EC, HD], F16)
                nc.sync.dma_start(
                    out=wqr_sb,
                    in_=wqr.ap().rearrange("(n p) f -> p n f", p=128))
                bdkv_sb = pA.tile([128, LC], F32)
                nc.sync.dma_start(
                    out=bdkv_sb, in_=bdkv.ap().rearrange("(n p) -> p n", p=128))
                bdq_sb = pA.tile([128, LC], F32)
                nc.sync.dma_start(
                    out=bdq_sb, in_=bdq.ap().rearrange("(n p) -> p n", p=128))
                bkr_sb = pA.tile([128, 1], F32)
                nc.sync.dma_start(
                    out=bkr_sb, in_=bkr.ap().rearrange("(n p) -> p n", p=128))
                bqr_sb = pA.tile([128, 1], F32)
                nc.sync.dma_start(
                    out=bqr_sb, in_=bqr.ap().rearrange("(n p) -> p n", p=128))
                pet_sb = pA.tile([128, TPC], F32)
                nc.sync.dma_start(out=pet_sb, in_=pet[:, :])

                # c_kvT shard -> bounce1 -> AllGather #1 (early, overlaps rest)
                ckvT_sb = pA.tile([128, LC, TPC], F16)
                for lc in range(LC):
                    ps = psA.tile([128, TPC], F32, tag="ps_a")
                    for ec in range(EC):
                        nc.tensor.matmul(
                            ps[:, :],
                            wdkv_sb[:, ec, lc * 128:(lc + 1) * 128],
                            x_sb[:, ec, :],
                            start=(ec == 0), stop=(ec == EC - 1))
                    nc.scalar.add(ckvT_sb[:, lc, :], ps[:, :],
                                  bdkv_sb[:, lc:lc + 1])
                nc.sync.dma_start(
                    out=bin1[0, :].rearrange("(n p f) -> p n f", p=128, f=TPC),
                    in_=ckvT_sb)
                if use_cc:
                    nc.gpsimd.collective_compute(
                        "AllGather", mybir.AluOpType.bypass,
                        replica_groups=[list(range(N_CORES))],
                        ins=[bin1.opt()], outs=[bout1.opt()])

                # c_qT / ropes -> bounce2 -> AllGather #2
                cqT_sb = pA.tile([128, LC, TPC], F16)
                for lc in range(LC):
                    ps = psA.tile([128, TPC], F32, tag="ps_a")
                    for ec in range(EC):
                        nc.tensor.matmul(
                            ps[:, :],
                            wdq_sb[:, ec, lc * 128:(lc + 1) * 128],
                            x_sb[:, ec, :],
                            start=(ec == 0), stop=(ec == EC - 1))
                    nc.scalar.add(cqT_sb[:, lc, :], ps[:, :],
                                  bdq_sb[:, lc:lc + 1])
                krT_sb = pA.tile([128, TPC], F16)
                ps = psA.tile([128, TPC], F32, tag="ps_a")
                for ec in range(EC):
                    nc.tensor.matmul(ps[:, :], wkr_sb[:, ec, :], x_sb[:, ec, :],
                                     start=(ec == 0), stop=(ec == EC - 1))
                tmpr = pA.tile([128, TPC], F32)
                nc.scalar.add(tmpr[:, :], ps[:, :], bkr_sb[:, 0:1])
                nc.vector.tensor_mul(krT_sb[:, :], tmpr[:, :], pet_sb[:, :])
                qrT_sb = pA.tile([128, TPC], F16)
                ps = psA.tile([128, TPC], F32, tag="ps_a")
                for ec in range(EC):
                    nc.tensor.matmul(ps[:, :], wqr_sb[:, ec, :], x_sb[:, ec, :],
                                     start=(ec == 0), stop=(ec == EC - 1))
                tmpr2 = pA.tile([128, TPC], F32)
                nc.scalar.add(tmpr2[:, :], ps[:, :], bqr_sb[:, 0:1])
                nc.vector.tensor_mul(qrT_sb[:, :], tmpr2[:, :], pet_sb[:, :])

                nc.sync.dma_start(
                    out=bin2[0, 0:KR_OFF].rearrange(
                        "(n p f) -> p n f", p=128, f=TPC),
                    in_=cqT_sb)
                nc.sync.dma_start(
                    out=bin2[0, KR_OFF:QR_OFF].rearrange(
                        "(p f) -> p f", p=128), in_=krT_sb)
                nc.sync.dma_start(
                    out=bin2[0, QR_OFF:AUX_N].rearrange(
                        "(p f) -> p f", p=128), in_=qrT_sb)
                if use_cc:
                    nc.gpsimd.collective_compute(
                        "AllGather", mybir.AluOpType.bypass,
                        replica_groups=[list(range(N_CORES))],
                        ins=[bin2.opt()], outs=[bout2.opt()])

            # ================= Phase B: up-projections =====================
            _pB_cm = tc.tile_pool(name="pB", bufs=1)
            pB = _pB_cm.__enter__()
            ckv_sb = pB.tile([128, LC, RB, TPC], F16)
            for r in range(RB):
                nc.sync.dma_start(
                    out=ckv_sb[:, :, r, :],
                    in_=bout1[r, :].rearrange("(n p f) -> p n f",
                                              p=128, f=TPC))
            kr_sb = pB.tile([128, RB, TPC], F16)
            nc.sync.dma_start(
                out=kr_sb,
                in_=bout2[:, KR_OFF:QR_OFF].rearrange(
                    "r (p f) -> p r f", p=128))
            qr_sb = pB.tile([128, RB, TPC], F16)
            nc.sync.dma_start(
                out=qr_sb,
                in_=bout2[:, QR_OFF:AUX_N].rearrange(
                    "r (p f) -> p r f", p=128))
            kc_sb = pB.tile([128, HPC, RB, TPC], F16)
            qc_sb = pB.tile([128, HPC, RB, TPC], F16)
            v_sb = pB.tile([128, HPC, TOK // 128, 132], F16)
            wo_sb = pB.tile([128, HPC, DIM], F16)
            nc.sync.dma_start(
                out=wo_sb, in_=wo.ap().rearrange("(n p) f -> p n f", p=128))
            buv_sb = pB.tile([128, HPC], F32)
            nc.sync.dma_start(
                out=buv_sb, in_=buv.ap().rearrange("(n p) -> p n", p=128))
            ident = pB.tile([128, 128], F16)
            make_identity(nc, ident)
            nc.vector.memset(v_sb[:, :, :, 128:129], 1.0)

            with tc.tile_pool(name="pQ", bufs=1) as pQ, \
                 tc.tile_pool(name="psU", bufs=2, space="PSUM") as psU:
                cq_sb = pQ.tile([128, LC, RB, TPC], F16)
                for r in range(RB):
                    nc.sync.dma_start(
                        out=cq_sb[:, :, r, :],
                        in_=bout2[r, 0:KR_OFF].rearrange(
                            "(n p f) -> p n f", p=128, f=TPC))
                wuk_sb = pQ.tile([128, LC, HPC * HD], F16)
                nc.sync.dma_start(
                    out=wuk_sb,
                    in_=wuk.ap().rearrange("(n p) f -> p n f", p=128))
                wuv_sb = pQ.tile([128, LC, HPC * HD], F16)
                nc.sync.dma_start(
                    out=wuv_sb,
                    in_=wuv.ap().rearrange("(n p) f -> p n f", p=128))
                wuq_sb = pQ.tile([128, LC, HPC * HD], F16)
                nc.sync.dma_start(
                    out=wuq_sb,
                    in_=wuq.ap().rearrange("(n p) f -> p n f", p=128))
                buk_sb = pQ.tile([128, HPC], F32)
                nc.sync.dma_start(
                    out=buk_sb, in_=buk.ap().rearrange("(n p) -> p n", p=128))
                buq_sb = pQ.tile([128, HPC], F32)
                nc.sync.dma_start(
                    out=buq_sb, in_=buq.ap().rearrange("(n p) -> p n", p=128))

                for h in range(HPC):
                    hs = slice(h * HD, (h + 1) * HD)
                    for rb in range(RB):
                        psk = psU.tile([128, TPC], F32, tag="ps_u")
                        for lc in range(LC):
                            nc.tensor.matmul(
                                psk[:, :], wuk_sb[:, lc, hs],
                                ckv_sb[:, lc, rb, :],
                                start=(lc == 0), stop=(lc == LC - 1))
                        nc.scalar.add(kc_sb[:, h, rb, :], psk[:, :],
                                      buk_sb[:, h:h + 1])
                        psq = psU.tile([128, TPC], F32, tag="ps_u")
                        for lc in range(LC):
                            nc.tensor.matmul(
                                psq[:, :], wuq_sb[:, lc, hs],
                                cq_sb[:, lc, rb, :],
                                start=(lc == 0), stop=(lc == LC - 1))
                        nc.scalar.add(qc_sb[:, h, rb, :], psq[:, :],
                                      buq_sb[:, h:h + 1])
                    for t in range(TOK // 128):
                        psv = psU.tile([128, 128], F32, tag="ps_v")
                        for lc in range(LC):
                            nc.tensor.matmul(
                                psv[:, :],
                                ckv_sb[:, lc, t // 4,
                                       (t % 4) * 128:(t % 4 + 1) * 128],
                                wuv_sb[:, lc, hs],
                                start=(lc == 0), stop=(lc == LC - 1))
                        nc.vector.tensor_copy(v_sb[:, h, t, 0:128], psv[:, :])

            # ================= Phase B: attention + output proj ============
            with tc.tile_pool(name="pAt", bufs=2) as pAt, \
                 tc.tile_pool(name="psS", bufs=o["psS_bufs"],
                              space="PSUM") as psS, \
                 tc.tile_pool(name="psC", bufs=o["psC_bufs"],
                              space="PSUM") as psC, \
                 tc.tile_pool(name="psT", bufs=o["psT_bufs"],
                              space="PSUM") as psT, \
                 tc.tile_pool(name="psO", bufs=o["psO_bufs"],
                              space="PSUM") as psO:
                for b in range(B):
                    for sb4 in range(NSB):
                        rq = 4 * b + sb4     # query rank-block
                        ctxT_sb = pAt.tile([128, HPC, SBLK // 128, 128], F16,
                                           tag="ctxT")
                        for h in range(HPC):
                            est_sb = pAt.tile([128, TC_B, SBLK], F16,
                                              tag="est", bufs=o["est_bufs"])
                            for tp in range(TC_B // 2):
                                # two key chunks share one wide PSUM tile so
                                # the exp runs over 1024 elems per ACT instr
                                ps_s = psS.tile([128, 2, SBLK], F32,
                                                tag="ps_s")
                                for ti in range(2):
                                    t = 2 * tp + ti
                                    rk = 4 * b + t // 4
                                    ko = (t % 4) * 128
                                    nc.tensor.matmul(
                                        ps_s[:, ti, :],
                                        kc_sb[:, h, rk, ko:ko + 128],
                                        qc_sb[:, h, rq, :],
                                        start=True, stop=False)
                                    nc.tensor.matmul(
                                        ps_s[:, ti, :],
                                        kr_sb[:, rk, ko:ko + 128],
                                        qr_sb[:, rq, :],
                                        start=False, stop=True)
                                nc.scalar.activation(
                                    est_sb[:, 2 * tp:2 * tp + 2, :],
                                    ps_s[:, :, :],
                                    mybir.ActivationFunctionType.Exp,
                                    scale=SCALE)
                            for sp in range(SBLK // 256):
                                ps_c = psC.tile([128, 2, 132], F32,
                                                tag="ps_c")
                                for si in range(2):
                                    ss = 2 * sp + si
                                    for t in range(TC_B):
                                        nc.tensor.matmul(
                                            ps_c[:, si, 0:129],
                                            est_sb[:, t,
                                                   ss * 128:(ss + 1) * 128],
                                            v_sb[:, h, TC_B * b + t, 0:129],
                                            start=(t == 0),
                                            stop=(t == TC_B - 1))
                                    recip = pAt.tile([128, 1], F32,
                                                     tag="recip")
                                    nc.vector.reciprocal(recip,
                                                         ps_c[:, si, 128:129])
                                    ctxn_sb = pAt.tile([128, 128], F16,
                                                       tag="ctxn")
                                    nc.vector.tensor_scalar_mul(
                                        ctxn_sb[:, :], ps_c[:, si, 0:128],
                                        recip)
                                    ps_t = psT.tile([128, 128], F16,
                                                    tag="ps_t")
                                    nc.tensor.transpose(ps_t[:, :],
                                                        ctxn_sb[:, :],
                                                        ident[:, :])
                                    nc.vector.tensor_scalar_add(
                                        ctxT_sb[:, h, ss, :], ps_t[:, :],
                                        buv_sb[:, h:h + 1])
                        n0 = (rq * TPC) // 128
                        out_dram = out_part.ap().rearrange(
                            "(n p) f -> p n f", p=128)
                        for ss in range(SBLK // 128):
                            out_sb = pAt.tile([128, DIM], F16, tag="out")
                            for dt4 in range(DIM // 512):
                                ps_o = psO.tile([128, 512], F32, tag="ps_o")
                                for h in range(HPC):
                                    nc.tensor.matmul(
                                        ps_o[:, :],
                                        ctxT_sb[:, h, ss, :],
                                        wo_sb[:, h, dt4 * 512:(dt4 + 1) * 512],
                                        start=(h == 0), stop=(h == HPC - 1))
                                nc.vector.tensor_copy(
                                    out_sb[:, dt4 * 512:(dt4 + 1) * 512],
                                    ps_o[:, :])
                            nc.sync.dma_start(
                                out=out_dram[:, n0 + ss, :], in_=out_sb)
            _pB_cm.__exit__(None, None, None)

    nc.compile()
    return nc


def _rope_pe():
    pos = np.arange(S, dtype=np.float32)[:, None]
    div = np.exp(np.arange(0, HD, 2, dtype=np.float32)
                 * (-math.log(10000.0) / HD))
    pe = np.zeros((S, HD), dtype=np.float32)
    pe[:, 0::2] = np.sin(pos * div)
    pe[:, 1::2] = np.cos(pos * div)
    return pe


def _prep_in_maps(inputs):
    f16 = np.float16
    x = np.asarray(inputs["x"], np.float32).reshape(TOK, DIM)
    pe = _rope_pe()
    shared = dict(
        wdkv=np.ascontiguousarray(np.asarray(inputs["W_DKV"], np.float32),
                                  dtype=f16),
        wdq=np.asarray(inputs["W_DQ"], np.float32).astype(f16),
        wkr=np.asarray(inputs["W_KR"], np.float32).astype(f16),
        wqr=np.asarray(inputs["W_QR"], np.float32).astype(f16),
        bdkv=np.asarray(inputs["b_DKV"], np.float32),
        bdq=np.asarray(inputs["b_DQ"], np.float32),
        bkr=np.asarray(inputs["b_KR"], np.float32),
        bqr=np.asarray(inputs["b_QR"], np.float32),
    )
    in_maps = []
    for r in range(N_CORES):
        tok = slice(r * TPC, (r + 1) * TPC)
        hslice = slice(r * HPC * HD, (r + 1) * HPC * HD)
        pos0 = (r * TPC) % S
        m = dict(shared)
        m["xT"] = np.ascontiguousarray(x[tok].T).astype(f16)
        m["pet"] = np.ascontiguousarray(pe[pos0:pos0 + TPC].T)
        m["wuk"] = np.ascontiguousarray(
            np.asarray(inputs["W_UK"], np.float32)[:, hslice]).astype(f16)
        m["wuv"] = np.ascontiguousarray(
            np.asarray(inputs["W_UV"], np.float32)[:, hslice]).astype(f16)
        m["wuq"] = np.ascontiguousarray(
            np.asarray(inputs["W_UQ"], np.float32)[:, hslice]).astype(f16)
        m["buk"] = np.ascontiguousarray(
            np.asarray(inputs["b_UK"], np.float32)[hslice])
        m["buv"] = np.ascontiguousarray(
            np.asarray(inputs["b_UV"], np.float32)[hslice])
        m["buq"] = np.ascontiguousarray(
            np.asarray(inputs["b_UQ"], np.float32)[hslice])
        m["wo"] = np.ascontiguousarray(
            np.asarray(inputs["W_O"], np.float32)[hslice, :]).astype(f16)
        in_maps.append(m)
    return in_maps


def _build_single(**opts):
    """Single-core, collective-free variant for cost-model timing."""
    return _build(use_cc=False, n_devices=1, **opts)


def _get_exec():
    """Build (once) a jitted shard_map executor over the 8 cores, mirroring
    concourse.bass2jax.run_bass_via_pjrt but cached so repeated kernel()
    calls do not re-trace/re-compile."""
    if "exec" in _CACHE:
        return _CACHE["exec"]
    import jax
    from jax.sharding import Mesh, PartitionSpec, NamedSharding
    from jax.experimental.shard_map import shard_map
    from concourse import bass2jax

    bass2jax.install_neuronx_cc_hook()
    if "nc" not in _CACHE:
        _CACHE["nc"] = _build()
    nc = _CACHE["nc"]

    _pname = nc.partition_id_tensor.name if nc.partition_id_tensor else None
    in_names, out_names, out_avals, zero_outs = [], [], [], []
    for alloc in nc.m.functions[0].allocations:
        if not isinstance(alloc, mybir.MemoryLocationSet):
            continue
        name = alloc.memorylocations[0].name
        if alloc.kind == "ExternalInput":
            if name != _pname:
                in_names.append(name)
        elif alloc.kind == "ExternalOutput":
            out_names.append(name)
            shape = tuple(alloc.tensor_shape)
            dtype = mybir.dt.np(alloc.dtype)
            out_avals.append(jax.core.ShapedArray(shape, dtype))
            zero_outs.append(np.zeros((N_CORES * shape[0], *shape[1:]), dtype))
    n_params = len(in_names)
    partition_name = (nc.partition_id_tensor.name
                      if nc.partition_id_tensor else None)
    all_names = in_names + out_names
    if partition_name is not None:
        all_names = all_names + [partition_name]

    def _body(*args):
        operands = list(args)
        if partition_name is not None:
            operands.append(bass2jax.partition_id_tensor())
        outs = bass2jax._bass_exec_p.bind(
            *operands,
            out_avals=tuple(out_avals),
            in_names=tuple(all_names),
            out_names=tuple(out_names),
            lowering_input_output_aliases=(),
            sim_require_finite=True,
            sim_require_nnan=True,
            nc=nc,
        )
        return tuple(outs)

    devices = jax.devices()[:N_CORES]
    mesh = Mesh(np.asarray(devices), ("core",))
    spec = PartitionSpec("core")
    in_specs = (spec,) * (n_params + len(out_names))
    out_specs = (spec,) * len(out_names)
    sharded = jax.jit(
        shard_map(_body, mesh=mesh, in_specs=in_specs, out_specs=out_specs,
                  check_rep=False),
        keep_unused=True,
    )
    sharding = NamedSharding(mesh, spec)
    zeros_dev = [jax.device_put(z, sharding) for z in zero_outs]
    _CACHE["exec"] = (sharded, in_names, out_names, out_avals, zeros_dev,
                      sharding)
    return _CACHE["exec"]


def _execute(in_maps):
    import jax
    sharded, in_names, out_names, out_avals, zeros_dev, sharding = _get_exec()
    concat_in = [
        np.concatenate([np.asarray(in_maps[c][n]) for c in range(N_CORES)],
                       axis=0)
        for n in in_names
    ]
    dev_in = [jax.device_put(a, sharding) for a in concat_in]
    out_arrs = sharded(*dev_in, *zeros_dev)
    out_arrs = [np.asarray(o) for o in out_arrs]
    return [
        {n: out_arrs[i].reshape(N_CORES, *out_avals[i].shape)[c]
         for i, n in enumerate(out_names)}
        for c in range(N_CORES)
    ]


def run(timeit=False, **inputs):
    in_maps = _prep_in_maps(inputs)
    results = _execute(in_maps)
    acc = np.zeros((TOK, DIM), np.float32)
    for r in range(N_CORES):
        acc += results[r]["out_part"].astype(np.float32)
    acc += np.asarray(inputs["b_O"], np.float32)
    return acc.reshape(B, S, DIM), results


def exec_only(in_maps):
    """For timing: run the prebuilt executor on preprocessed inputs."""
    return _execute(in_maps)


def timeit(inputs, n=10):
    """Time the device execution with device-resident inputs (excludes
    host prep and H2D transfer; includes PJRT/tunnel dispatch)."""
    import time
    import jax
    in_maps = _prep_in_maps(inputs)
    sharded, in_names, _, _, zeros_dev, sharding = _get_exec()
    dev_in = [
        jax.device_put(
            np.concatenate([np.asarray(in_maps[c][nm])
                            for c in range(N_CORES)], axis=0), sharding)
        for nm in in_names
    ]
    outs = sharded(*dev_in, *zeros_dev)   # warm-up
    jax.block_until_ready(outs)
    times = []
    for _ in range(n):
        t0 = time.perf_counter()
        outs = sharded(*dev_in, *zeros_dev)
        jax.block_until_ready(outs)
        times.append(time.perf_counter() - t0)
    return times


def kernel(**inputs):
    out, _ = run(**inputs)
    return out


# revision 39
# speedup vs baseline: 1.3851x; 1.3851x over previous
"""Multi-Head Latent Attention (MLA) prefill kernel for 8 Trainium2 NeuronCores.

Problem shapes: B=2, S=2048, DIM=2048, H=16 heads, HEAD_DIM=128, LATENT=512.

Strategy (tensor-parallel over heads + data-parallel phase A):
  Phase A (token-DP): each core computes the latent down-projections
    c_kvT, c_qT and the rope projections k_rT, q_rT (pe-scaled) for its
    512-token shard, in transposed [feature, token] layout, fp16.
  AllGather (x2, overlapped): shards are exchanged so every core holds the
    full-sequence latents/ropes (~3.4MB/rank fp16 total).
  Phase B (head-TP): each core serves 2 of the 16 heads: up-projects
    k_c/q_c (transposed) and v (natural, with a ones-column appended so the
    softmax denominator falls out of the ctx matmul for free), then runs
    attention (scores^T tiles -> exp on ACT -> ctx accumulate in PSUM ->
    per-row normalize -> PE transpose -> W_O row-slice partial product).
  Host: sums the 8 partial outputs and adds b_O.

All matmuls run with fp16 operands and fp32 PSUM accumulation
(measured end-to-end rel. error ~3e-4 in numpy simulation).
"""
import math

import numpy as np

import concourse.bacc as bacc
import concourse.mybir as mybir
import concourse.tile as tile
from concourse import bass_utils
from concourse.masks import make_identity

# Problem constants (hardcoded per harness contract).
B, S, DIM, H, HD, LAT = 2, 2048, 2048, 16, 128, 512
N_CORES = 8
HPC = H // N_CORES          # heads per core = 2
TOK = B * S                 # 4096 tokens
TPC = TOK // N_CORES        # 512 tokens per core (phase A shard)
EC = DIM // 128             # 16 embedding chunks
LC = LAT // 128             # 4 latent chunks
RB = N_CORES                # rank blocks of TPC tokens
SBLK = 512                  # query block (phase B)
NSB = S // SBLK             # 4 s-blocks per batch
TC_B = S // 128             # 16 key chunks per batch
F16 = mybir.dt.float16
F32 = mybir.dt.float32
SCALE = 1.0 / math.sqrt(HD)

_CACHE = {}


OPTS = dict(psS_bufs=2, psC_bufs=1, psT_bufs=1, psO_bufs=2, est_bufs=2,
            skip_attn=False, skip_ctx=False)


def _build(use_cc=True, n_devices=N_CORES, **opts):
    o = dict(OPTS)
    o.update(opts)
    nc = bacc.Bacc("TRN2", target_bir_lowering=False, debug=False,
                   num_devices=n_devices)

    # ---- per-core external inputs (host pre-permuted into SBUF layout so
    # every input DMA is a contiguous [128, X] copy) ----
    xT = nc.dram_tensor("xT", [128, EC, TPC], F16, kind="ExternalInput")
    wdkv = nc.dram_tensor("wdkv", [128, EC, LAT], F16, kind="ExternalInput")
    wdq = nc.dram_tensor("wdq", [128, EC, LAT], F16, kind="ExternalInput")
    wkr = nc.dram_tensor("wkr", [128, EC, HD], F16, kind="ExternalInput")
    wqr = nc.dram_tensor("wqr", [128, EC, HD], F16, kind="ExternalInput")
    bdkv = nc.dram_tensor("bdkv", [128, LC], F32, kind="ExternalInput")
    bdq = nc.dram_tensor("bdq", [128, LC], F32, kind="ExternalInput")
    bkr = nc.dram_tensor("bkr", [128, 1], F32, kind="ExternalInput")
    bqr = nc.dram_tensor("bqr", [128, 1], F32, kind="ExternalInput")
    pet = nc.dram_tensor("pet", [HD, TPC], F32, kind="ExternalInput")
    wuk = nc.dram_tensor("wuk", [128, LC, HPC * HD], F16,
                         kind="ExternalInput")
    wuv = nc.dram_tensor("wuv", [128, LC, HPC * HD], F16,
                         kind="ExternalInput")
    wuq = nc.dram_tensor("wuq", [128, LC, HPC * HD], F16,
                         kind="ExternalInput")
    buk = nc.dram_tensor("buk", [128, HPC], F32, kind="ExternalInput")
    buv = nc.dram_tensor("buv", [128, HPC], F32, kind="ExternalInput")
    buq = nc.dram_tensor("buq", [128, HPC], F32, kind="ExternalInput")
    wo = nc.dram_tensor("wo", [128, HPC, DIM], F16, kind="ExternalInput")

    out_part = nc.dram_tensor("out_part", [TOK, DIM], F16,
                              kind="ExternalOutput")

    CKV_N = LAT * TPC            # 262144 elems per rank shard
    AUX_N = LAT * TPC + 2 * HD * TPC  # c_qT + k_rT + q_rT = 393216
    KR_OFF = LAT * TPC
    QR_OFF = KR_OFF + HD * TPC

    with tile.TileContext(nc) as tc:
        with tc.tile_pool(name="dram", bufs=1, space="DRAM") as dram:
            bin1 = dram.tile([1, CKV_N], F16)
            bout1 = dram.tile([RB, CKV_N], F16, addr_space="Shared")
            bin2 = dram.tile([1, AUX_N], F16)
            bout2 = dram.tile([RB, AUX_N], F16, addr_space="Shared")

            # ================= Phase A (token shard, transposed outputs) ===
            with tc.tile_pool(name="pA", bufs=1) as pA, \
                 tc.tile_pool(name="psA", bufs=2, space="PSUM") as psA:
                x_sb = pA.tile([128, EC, TPC], F16)
                nc.sync.dma_start(out=x_sb[:, 0:4], in_=xT[:, 0:4, :])
                nc.sync.dma_start(out=x_sb[:, 4:EC], in_=xT[:, 4:EC, :])
                wdkv_sb = pA.tile([128, EC, LAT], F16)
                nc.sync.dma_start(out=wdkv_sb[:, 0:4], in_=wdkv[:, 0:4, :])
                nc.sync.dma_start(out=wdkv_sb[:, 4:EC], in_=wdkv[:, 4:EC, :])
                wdq_sb = pA.tile([128, EC, LAT], F16)
                nc.sync.dma_start(out=wdq_sb, in_=wdq[:, :, :])
                wkr_sb = pA.tile([128, EC, HD], F16)
                nc.sync.dma_start(out=wkr_sb, in_=wkr[:, :, :])
                wqr_sb = pA.tile([128, EC, HD], F16)
                nc.sync.dma_start(out=wqr_sb, in_=wqr[:, :, :])
                bdkv_sb = pA.tile([128, LC], F32)
                nc.sync.dma_start(out=bdkv_sb, in_=bdkv[:, :])
                bdq_sb = pA.tile([128, LC], F32)
                nc.sync.dma_start(out=bdq_sb, in_=bdq[:, :])
                bkr_sb = pA.tile([128, 1], F32)
                nc.sync.dma_start(out=bkr_sb, in_=bkr[:, :])
                bqr_sb = pA.tile([128, 1], F32)
                nc.sync.dma_start(out=bqr_sb, in_=bqr[:, :])
                pet_sb = pA.tile([128, TPC], F32)
                nc.sync.dma_start(out=pet_sb, in_=pet[:, :])

                # c_kvT shard -> bounce1 -> AllGather #1 (early, overlaps rest)
                ckvT_sb = pA.tile([128, LC, TPC], F16)
                for lc in range(LC):
                    ps = psA.tile([128, TPC], F32, tag="ps_a")
                    for ec in range(EC):
                        nc.tensor.matmul(
                            ps[:, :],
                            wdkv_sb[:, ec, lc * 128:(lc + 1) * 128],
                            x_sb[:, ec, :],
                            start=(ec == 0), stop=(ec == EC - 1))
                    nc.scalar.add(ckvT_sb[:, lc, :], ps[:, :],
                                  bdkv_sb[:, lc:lc + 1])
                nc.sync.dma_start(
                    out=bin1[0, :].rearrange("(p n f) -> p n f", p=128, f=TPC),
                    in_=ckvT_sb)
                if use_cc:
                    nc.gpsimd.collective_compute(
                        "AllGather", mybir.AluOpType.bypass,
                        replica_groups=[list(range(N_CORES))],
                        ins=[bin1.opt()], outs=[bout1.opt()])

                # c_qT / ropes -> bounce2 -> AllGather #2
                cqT_sb = pA.tile([128, LC, TPC], F16)
                for lc in range(LC):
                    ps = psA.tile([128, TPC], F32, tag="ps_a")
                    for ec in range(EC):
                        nc.tensor.matmul(
                            ps[:, :],
                            wdq_sb[:, ec, lc * 128:(lc + 1) * 128],
                            x_sb[:, ec, :],
                            start=(ec == 0), stop=(ec == EC - 1))
                    nc.scalar.add(cqT_sb[:, lc, :], ps[:, :],
                                  bdq_sb[:, lc:lc + 1])
                krT_sb = pA.tile([128, TPC], F16)
                ps = psA.tile([128, TPC], F32, tag="ps_a")
                for ec in range(EC):
                    nc.tensor.matmul(ps[:, :], wkr_sb[:, ec, :], x_sb[:, ec, :],
                                     start=(ec == 0), stop=(ec == EC - 1))
                tmpr = pA.tile([128, TPC], F32)
                nc.scalar.add(tmpr[:, :], ps[:, :], bkr_sb[:, 0:1])
                nc.vector.tensor_mul(krT_sb[:, :], tmpr[:, :], pet_sb[:, :])
                qrT_sb = pA.tile([128, TPC], F16)
                ps = psA.tile([128, TPC], F32, tag="ps_a")
                for ec in range(EC):
                    nc.tensor.matmul(ps[:, :], wqr_sb[:, ec, :], x_sb[:, ec, :],
                                     start=(ec == 0), stop=(ec == EC - 1))
                tmpr2 = pA.tile([128, TPC], F32)
                nc.scalar.add(tmpr2[:, :], ps[:, :], bqr_sb[:, 0:1])
                nc.vector.tensor_mul(qrT_sb[:, :], tmpr2[:, :], pet_sb[:, :])

                nc.sync.dma_start(
                    out=bin2[0, 0:KR_OFF].rearrange(
                        "(p n f) -> p n f", p=128, f=TPC),
                    in_=cqT_sb)
                nc.sync.dma_start(
                    out=bin2[0, KR_OFF:QR_OFF].rearrange(
                        "(p f) -> p f", p=128), in_=krT_sb)
                nc.sync.dma_start(
                    out=bin2[0, QR_OFF:AUX_N].rearrange(
                        "(p f) -> p f", p=128), in_=qrT_sb)
                if use_cc:
                    nc.gpsimd.collective_compute(
                        "AllGather", mybir.AluOpType.bypass,
                        replica_groups=[list(range(N_CORES))],
                        ins=[bin2.opt()], outs=[bout2.opt()])

            # ================= Phase B: up-projections =====================
            _pB_cm = tc.tile_pool(name="pB", bufs=1)
            pB = _pB_cm.__enter__()
            ckv_sb = pB.tile([128, LC, RB, TPC], F16)
            for r in range(RB):
                nc.sync.dma_start(
                    out=ckv_sb[:, :, r, :],
                    in_=bout1[r, :].rearrange("(p n f) -> p n f",
                                              p=128, f=TPC))
            kr_sb = pB.tile([128, RB, TPC], F16)
            nc.sync.dma_start(
                out=kr_sb,
                in_=bout2[:, KR_OFF:QR_OFF].rearrange(
                    "r (p f) -> p r f", p=128))
            qr_sb = pB.tile([128, RB, TPC], F16)
            nc.sync.dma_start(
                out=qr_sb,
                in_=bout2[:, QR_OFF:AUX_N].rearrange(
                    "r (p f) -> p r f", p=128))
            kc_sb = pB.tile([128, HPC, RB, TPC], F16)
            qc_sb = pB.tile([128, HPC, RB, TPC], F16)
            v_sb = pB.tile([128, HPC, TOK // 128, 132], F16)
            wo_sb = pB.tile([128, HPC, DIM], F16)
            nc.sync.dma_start(out=wo_sb, in_=wo[:, :, :])
            buv_sb = pB.tile([128, HPC], F32)
            nc.sync.dma_start(out=buv_sb, in_=buv[:, :])
            ident = pB.tile([128, 128], F16)
            make_identity(nc, ident)
            nc.vector.memset(v_sb[:, :, :, 128:129], 1.0)

            with tc.tile_pool(name="pQ", bufs=1) as pQ, \
                 tc.tile_pool(name="psU", bufs=2, space="PSUM") as psU:
                cq_sb = pQ.tile([128, LC, RB, TPC], F16)
                for r in range(RB):
                    nc.sync.dma_start(
                        out=cq_sb[:, :, r, :],
                        in_=bout2[r, 0:KR_OFF].rearrange(
                            "(p n f) -> p n f", p=128, f=TPC))
                wuk_sb = pQ.tile([128, LC, HPC * HD], F16)
                nc.sync.dma_start(out=wuk_sb, in_=wuk[:, :, :])
                wuv_sb = pQ.tile([128, LC, HPC * HD], F16)
                nc.sync.dma_start(out=wuv_sb, in_=wuv[:, :, :])
                wuq_sb = pQ.tile([128, LC, HPC * HD], F16)
                nc.sync.dma_start(out=wuq_sb, in_=wuq[:, :, :])
                buk_sb = pQ.tile([128, HPC], F32)
                nc.sync.dma_start(out=buk_sb, in_=buk[:, :])
                buq_sb = pQ.tile([128, HPC], F32)
                nc.sync.dma_start(out=buq_sb, in_=buq[:, :])

                for h in range(HPC):
                    hs = slice(h * HD, (h + 1) * HD)
                    for rb in range(RB):
                        psk = psU.tile([128, TPC], F32, tag="ps_u")
                        for lc in range(LC):
                            nc.tensor.matmul(
                                psk[:, :], wuk_sb[:, lc, hs],
                                ckv_sb[:, lc, rb, :],
                                start=(lc == 0), stop=(lc == LC - 1))
                        nc.scalar.add(kc_sb[:, h, rb, :], psk[:, :],
                                      buk_sb[:, h:h + 1])
                        psq = psU.tile([128, TPC], F32, tag="ps_u")
                        for lc in range(LC):
                            nc.tensor.matmul(
                                psq[:, :], wuq_sb[:, lc, hs],
                                cq_sb[:, lc, rb, :],
                                start=(lc == 0), stop=(lc == LC - 1))
                        nc.scalar.add(qc_sb[:, h, rb, :], psq[:, :],
                                      buq_sb[:, h:h + 1])
                    for t in range(TOK // 128):
                        psv = psU.tile([128, 128], F32, tag="ps_v")
                        for lc in range(LC):
                            nc.tensor.matmul(
                                psv[:, :],
                                ckv_sb[:, lc, t // 4,
                                       (t % 4) * 128:(t % 4 + 1) * 128],
                                wuv_sb[:, lc, hs],
                                start=(lc == 0), stop=(lc == LC - 1))
                        nc.vector.tensor_copy(v_sb[:, h, t, 0:128], psv[:, :])

            # ================= Phase B: attention + output proj ============
            with tc.tile_pool(name="pAt", bufs=2) as pAt, \
                 tc.tile_pool(name="psS", bufs=o["psS_bufs"],
                              space="PSUM") as psS, \
                 tc.tile_pool(name="psC", bufs=o["psC_bufs"],
                              space="PSUM") as psC, \
                 tc.tile_pool(name="psT", bufs=o["psT_bufs"],
                              space="PSUM") as psT, \
                 tc.tile_pool(name="psO", bufs=o["psO_bufs"],
                              space="PSUM") as psO:
                for b in range(B):
                    if o["skip_attn"]:
                        break
                    for sb4 in range(NSB):
                        rq = 4 * b + sb4     # query rank-block
                        ctxT_sb = pAt.tile([128, HPC, SBLK // 128, 128], F16,
                                           tag="ctxT")
                        for h in range(HPC):
                            est_sb = pAt.tile([128, TC_B, SBLK], F16,
                                              tag="est", bufs=o["est_bufs"])
                            for tp in range(TC_B // 2):
                                # two key chunks share one wide PSUM tile so
                                # the exp runs over 1024 elems per ACT instr
                                ps_s = psS.tile([128, 2, SBLK], F32,
                                                tag="ps_s")
                                for ti in range(2):
                                    t = 2 * tp + ti
                                    rk = 4 * b + t // 4
                                    ko = (t % 4) * 128
                                    nc.tensor.matmul(
                                        ps_s[:, ti, :],
                                        kc_sb[:, h, rk, ko:ko + 128],
                                        qc_sb[:, h, rq, :],
                                        start=True, stop=False)
                                    nc.tensor.matmul(
                                        ps_s[:, ti, :],
                                        kr_sb[:, rk, ko:ko + 128],
                                        qr_sb[:, rq, :],
                                        start=False, stop=True)
                                nc.scalar.activation(
                                    est_sb[:, 2 * tp:2 * tp + 2, :],
                                    ps_s[:, :, :],
                                    mybir.ActivationFunctionType.Exp,
                                    scale=SCALE)
                            for sp in range(SBLK // 256):
                                if o["skip_ctx"]:
                                    break
                                ps_c = psC.tile([128, 2, 132], F32,
                                                tag="ps_c")
                                for si in range(2):
                                    ss = 2 * sp + si
                                    for t in range(TC_B):
                                        nc.tensor.matmul(
                                            ps_c[:, si, 0:129],
                                            est_sb[:, t,
                                                   ss * 128:(ss + 1) * 128],
                                            v_sb[:, h, TC_B * b + t, 0:129],
                                            start=(t == 0),
                                            stop=(t == TC_B - 1))
                                    recip = pAt.tile([128, 1], F32,
                                                     tag="recip")
                                    nc.vector.reciprocal(recip,
                                                         ps_c[:, si, 128:129])
                                    ctxn_sb = pAt.tile([128, 128], F16,
                                                       tag="ctxn")
                                    nc.vector.tensor_scalar_mul(
                                        ctxn_sb[:, :], ps_c[:, si, 0:128],
                                        recip)
                                    ps_t = psT.tile([128, 128], F16,
                                                    tag="ps_t")
                                    nc.tensor.transpose(ps_t[:, :],
                                                        ctxn_sb[:, :],
                                                        ident[:, :])
                                    nc.vector.tensor_scalar_add(
                                        ctxT_sb[:, h, ss, :], ps_t[:, :],
                                        buv_sb[:, h:h + 1])
                        n0 = (rq * TPC) // 128
                        out_dram = out_part.ap().rearrange(
                            "(n p) f -> p n f", p=128)
                        for ss in range(SBLK // 128):
                            if o["skip_ctx"]:
                                break
                            out_sb = pAt.tile([128, DIM], F16, tag="out")
                            for dt4 in range(DIM // 512):
                                ps_o = psO.tile([128, 512], F32, tag="ps_o")
                                for h in range(HPC):
                                    nc.tensor.matmul(
                                        ps_o[:, :],
                                        ctxT_sb[:, h, ss, :],
                                        wo_sb[:, h, dt4 * 512:(dt4 + 1) * 512],
                                        start=(h == 0), stop=(h == HPC - 1))
                                nc.vector.tensor_copy(
                                    out_sb[:, dt4 * 512:(dt4 + 1) * 512],
                                    ps_o[:, :])
                            nc.sync.dma_start(
                                out=out_dram[:, n0 + ss, :], in_=out_sb)
            _pB_cm.__exit__(None, None, None)

    nc.compile()
    return nc


def _rope_pe():
    pos = np.arange(S, dtype=np.float32)[:, None]
    div = np.exp(np.arange(0, HD, 2, dtype=np.float32)
                 * (-math.log(10000.0) / HD))
    pe = np.zeros((S, HD), dtype=np.float32)
    pe[:, 0::2] = np.sin(pos * div)
    pe[:, 1::2] = np.cos(pos * div)
    return pe


def _sbl(w, f16=True):
    """[n*128, C...] -> SBUF layout [128, n, C...] (partition-major)."""
    w = np.asarray(w, np.float32)
    n = w.shape[0] // 128
    out = np.ascontiguousarray(
        w.reshape(n, 128, *w.shape[1:]).swapaxes(0, 1))
    return out.astype(np.float16) if f16 else out


def _sblb(b):
    """bias [n*128] -> [128, n] fp32."""
    b = np.asarray(b, np.float32)
    n = b.size // 128
    return np.ascontiguousarray(b.reshape(n, 128).T)


def _prep_in_maps(inputs):
    f16 = np.float16
    x = np.asarray(inputs["x"], np.float32).reshape(TOK, DIM)
    pe = _rope_pe()
    shared = dict(
        wdkv=_sbl(inputs["W_DKV"]),
        wdq=_sbl(inputs["W_DQ"]),
        wkr=_sbl(inputs["W_KR"]),
        wqr=_sbl(inputs["W_QR"]),
        bdkv=_sblb(inputs["b_DKV"]),
        bdq=_sblb(inputs["b_DQ"]),
        bkr=_sblb(inputs["b_KR"]),
        bqr=_sblb(inputs["b_QR"]),
    )
    in_maps = []
    for r in range(N_CORES):
        tok = slice(r * TPC, (r + 1) * TPC)
        hslice = slice(r * HPC * HD, (r + 1) * HPC * HD)
        pos0 = (r * TPC) % S
        m = dict(shared)
        # xT sbuf layout: [128, EC, TPC]; x_sb[p, n, f] = x[tok_f, n*128+p]
        m["xT"] = np.ascontiguousarray(
            x[tok].T.reshape(EC, 128, TPC).swapaxes(0, 1)).astype(f16)
        m["pet"] = np.ascontiguousarray(pe[pos0:pos0 + TPC].T)
        m["wuk"] = _sbl(np.asarray(inputs["W_UK"], np.float32)[:, hslice])
        m["wuv"] = _sbl(np.asarray(inputs["W_UV"], np.float32)[:, hslice])
        m["wuq"] = _sbl(np.asarray(inputs["W_UQ"], np.float32)[:, hslice])
        m["buk"] = _sblb(np.asarray(inputs["b_UK"], np.float32)[hslice])
        m["buv"] = _sblb(np.asarray(inputs["b_UV"], np.float32)[hslice])
        m["buq"] = _sblb(np.asarray(inputs["b_UQ"], np.float32)[hslice])
        m["wo"] = _sbl(np.asarray(inputs["W_O"], np.float32)[hslice, :])
        in_maps.append(m)
    return in_maps


def _build_single(**opts):
    """Single-core, collective-free variant for cost-model timing."""
    return _build(use_cc=False, n_devices=1, **opts)


def _get_exec():
    """Build (once) a jitted shard_map executor over the 8 cores, mirroring
    concourse.bass2jax.run_bass_via_pjrt but cached so repeated kernel()
    calls do not re-trace/re-compile."""
    if "exec" in _CACHE:
        return _CACHE["exec"]
    import jax
    from jax.sharding import Mesh, PartitionSpec, NamedSharding
    from jax.experimental.shard_map import shard_map
    from concourse import bass2jax

    bass2jax.install_neuronx_cc_hook()
    if "nc" not in _CACHE:
        _CACHE["nc"] = _build()
    nc = _CACHE["nc"]

    _pname = nc.partition_id_tensor.name if nc.partition_id_tensor else None
    in_names, out_names, out_avals, zero_outs = [], [], [], []
    for alloc in nc.m.functions[0].allocations:
        if not isinstance(alloc, mybir.MemoryLocationSet):
            continue
        name = alloc.memorylocations[0].name
        if alloc.kind == "ExternalInput":
            if name != _pname:
                in_names.append(name)
        elif alloc.kind == "ExternalOutput":
            out_names.append(name)
            shape = tuple(alloc.tensor_shape)
            dtype = mybir.dt.np(alloc.dtype)
            out_avals.append(jax.core.ShapedArray(shape, dtype))
            zero_outs.append(np.zeros((N_CORES * shape[0], *shape[1:]), dtype))
    n_params = len(in_names)
    partition_name = (nc.partition_id_tensor.name
                      if nc.partition_id_tensor else None)
    all_names = in_names + out_names
    if partition_name is not None:
        all_names = all_names + [partition_name]

    def _body(*args):
        operands = list(args)
        if partition_name is not None:
            operands.append(bass2jax.partition_id_tensor())
        outs = bass2jax._bass_exec_p.bind(
            *operands,
            out_avals=tuple(out_avals),
            in_names=tuple(all_names),
            out_names=tuple(out_names),
            lowering_input_output_aliases=(),
            sim_require_finite=True,
            sim_require_nnan=True,
            nc=nc,
        )
        return tuple(outs)

    devices = jax.devices()[:N_CORES]
    mesh = Mesh(np.asarray(devices), ("core",))
    spec = PartitionSpec("core")
    in_specs = (spec,) * (n_params + len(out_names))
    out_specs = (spec,) * len(out_names)
    sharded = jax.jit(
        shard_map(_body, mesh=mesh, in_specs=in_specs, out_specs=out_specs,
                  check_rep=False),
        keep_unused=True,
    )
    sharding = NamedSharding(mesh, spec)
    zeros_dev = [jax.device_put(z, sharding) for z in zero_outs]
    _CACHE["exec"] = (sharded, in_names, out_names, out_avals, zeros_dev,
                      sharding)
    return _CACHE["exec"]


def _execute(in_maps):
    import jax
    sharded, in_names, out_names, out_avals, zeros_dev, sharding = _get_exec()
    concat_in = [
        np.concatenate([np.asarray(in_maps[c][n]) for c in range(N_CORES)],
                       axis=0)
        for n in in_names
    ]
    dev_in = [jax.device_put(a, sharding) for a in concat_in]
    out_arrs = sharded(*dev_in, *zeros_dev)
    out_arrs = [np.asarray(o) for o in out_arrs]
    return [
        {n: out_arrs[i].reshape(N_CORES, *out_avals[i].shape)[c]
         for i, n in enumerate(out_names)}
        for c in range(N_CORES)
    ]


def run(timeit=False, **inputs):
    in_maps = _prep_in_maps(inputs)
    results = _execute(in_maps)
    acc = np.zeros((TOK, DIM), np.float32)
    for r in range(N_CORES):
        acc += results[r]["out_part"].astype(np.float32)
    acc += np.asarray(inputs["b_O"], np.float32)
    return acc.reshape(B, S, DIM), results


def exec_only(in_maps):
    """For timing: run the prebuilt executor on preprocessed inputs."""
    return _execute(in_maps)


def timeit_loop(inputs, reps=20, n=6):
    """Chain `reps` kernel executions inside one jitted dispatch (the output
    zero-buffer is threaded as the carry, forcing sequential execution), so
    (T(reps) - T(1)) / (reps - 1) isolates per-execution device time from
    the tunnel/dispatch floor."""
    import time
    import jax
    from jax.sharding import Mesh, PartitionSpec, NamedSharding
    from jax.experimental.shard_map import shard_map
    from concourse import bass2jax

    bass2jax.install_neuronx_cc_hook()
    if "nc" not in _CACHE:
        _CACHE["nc"] = _build()
    nc = _CACHE["nc"]
    _pname = nc.partition_id_tensor.name if nc.partition_id_tensor else None
    in_names, out_names, out_avals, zero_outs = [], [], [], []
    for alloc in nc.m.functions[0].allocations:
        if not isinstance(alloc, mybir.MemoryLocationSet):
            continue
        name = alloc.memorylocations[0].name
        if alloc.kind == "ExternalInput":
            if name != _pname:
                in_names.append(name)
        elif alloc.kind == "ExternalOutput":
            out_names.append(name)
            shape = tuple(alloc.tensor_shape)
            dtype = mybir.dt.np(alloc.dtype)
            out_avals.append(jax.core.ShapedArray(shape, dtype))
            zero_outs.append(np.zeros((N_CORES * shape[0], *shape[1:]),
                                      dtype))
    all_names = in_names + out_names + ([_pname] if _pname else [])

    def _exec_once(args, carry):
        operands = list(args) + [carry]
        if _pname is not None:
            operands.append(bass2jax.partition_id_tensor())
        outs = bass2jax._bass_exec_p.bind(
            *operands, out_avals=tuple(out_avals),
            in_names=tuple(all_names), out_names=tuple(out_names),
            lowering_input_output_aliases=(),
            sim_require_finite=True, sim_require_nnan=True, nc=nc)
        return outs[0]

    def make_body(r):
        def _body(*args):
            carry = args[-1]
            ins = args[:-1]
            for _ in range(r):
                carry = _exec_once(ins, carry)
            return (carry,)
        return _body

    devices = jax.devices()[:N_CORES]
    mesh = Mesh(np.asarray(devices), ("core",))
    spec = PartitionSpec("core")
    sharding = NamedSharding(mesh, spec)
    in_maps = _prep_in_maps(inputs)
    dev_in = [
        jax.device_put(
            np.concatenate([np.asarray(in_maps[c][nm])
                            for c in range(N_CORES)], axis=0), sharding)
        for nm in in_names
    ]
    dev_z = jax.device_put(zero_outs[0], sharding)

    results = {}
    for r in (1, reps):
        f = jax.jit(
            shard_map(make_body(r), mesh=mesh,
                      in_specs=(spec,) * (len(in_names) + 1),
                      out_specs=(spec,), check_rep=False),
            keep_unused=True)
        outs = f(*dev_in, dev_z)
        jax.block_until_ready(outs)
        best = float("inf")
        for _ in range(n):
            t0 = time.perf_counter()
            outs = f(*dev_in, dev_z)
            jax.block_until_ready(outs)
            best = min(best, time.perf_counter() - t0)
        results[r] = best
    per_exec = (results[reps] - results[1]) / (reps - 1)
    return per_exec, results


def timeit(inputs, n=10):
    """Time the device execution with device-resident inputs (excludes
    host prep and H2D transfer; includes PJRT/tunnel dispatch)."""
    import time
    import jax
    in_maps = _prep_in_maps(inputs)
    sharded, in_names, _, _, zeros_dev, sharding = _get_exec()
    dev_in = [
        jax.device_put(
            np.concatenate([np.asarray(in_maps[c][nm])
                            for c in range(N_CORES)], axis=0), sharding)
        for nm in in_names
    ]
    outs = sharded(*dev_in, *zeros_dev)   # warm-up
    jax.block_until_ready(outs)
    times = []
    for _ in range(n):
        t0 = time.perf_counter()
        outs = sharded(*dev_in, *zeros_dev)
        jax.block_until_ready(outs)
        times.append(time.perf_counter() - t0)
    return times


def kernel(**inputs):
    out, _ = run(**inputs)
    return out


# revision 45
# speedup vs baseline: 107.6300x; 77.7046x over previous
"""Multi-Head Latent Attention (MLA) prefill kernel for 8 Trainium2 NeuronCores.

Problem shapes: B=2, S=2048, DIM=2048, H=16 heads, HEAD_DIM=128, LATENT=512.

Strategy (tensor-parallel over heads + data-parallel phase A):
  Phase A (token-DP): each core computes the latent down-projections
    c_kvT, c_qT and the rope projections k_rT, q_rT (pe-scaled) for its
    512-token shard, in transposed [feature, token] layout, fp16.
  AllGather (x2, overlapped): shards are exchanged so every core holds the
    full-sequence latents/ropes (~3.4MB/rank fp16 total).
  Phase B (head-TP): each core serves 2 of the 16 heads: up-projects
    k_c/q_c (transposed) and v (natural, with a ones-column appended so the
    softmax denominator falls out of the ctx matmul for free), then runs
    attention (scores^T tiles -> exp on ACT -> ctx accumulate in PSUM ->
    per-row normalize -> PE transpose -> W_O row-slice partial product).
  Host: sums the 8 partial outputs and adds b_O.

All matmuls run with fp16 operands and fp32 PSUM accumulation
(measured end-to-end rel. error ~3e-4 in numpy simulation).
"""
import math

import numpy as np

import concourse.bacc as bacc
import concourse.mybir as mybir
import concourse.tile as tile
from concourse.masks import make_identity

# Problem constants (hardcoded per harness contract).
B, S, DIM, H, HD, LAT = 2, 2048, 2048, 16, 128, 512
N_CORES = 8
HPC = H // N_CORES          # heads per core = 2
TOK = B * S                 # 4096 tokens
TPC = TOK // N_CORES        # 512 tokens per core (phase A shard)
EC = DIM // 128             # 16 embedding chunks
LC = LAT // 128             # 4 latent chunks
RB = N_CORES                # rank blocks of TPC tokens
SBLK = 512                  # query block (phase B)
NSB = S // SBLK             # 4 s-blocks per batch
TC_B = S // 128             # 16 key chunks per batch
F16 = mybir.dt.float16
F32 = mybir.dt.float32
SCALE = 1.0 / math.sqrt(HD)

_CACHE = {}


OPTS = dict(psS_bufs=2, psC_bufs=1, psT_bufs=1, psO_bufs=2, est_bufs=2,
            skip_attn=False, skip_ctx=False, wide_exp=True)


def _build(use_cc=True, n_devices=N_CORES, **opts):
    o = dict(OPTS)
    o.update(opts)
    nc = bacc.Bacc("TRN2", target_bir_lowering=False, debug=False,
                   num_devices=n_devices)

    # ---- per-core external inputs (host pre-permuted into SBUF layout so
    # every input DMA is a contiguous [128, X] copy) ----
    xT = nc.dram_tensor("xT", [128, EC, TPC], F16, kind="ExternalInput")
    wdkv = nc.dram_tensor("wdkv", [128, EC, LAT], F16, kind="ExternalInput")
    wdq = nc.dram_tensor("wdq", [128, EC, LAT], F16, kind="ExternalInput")
    wkr = nc.dram_tensor("wkr", [128, EC, HD], F16, kind="ExternalInput")
    wqr = nc.dram_tensor("wqr", [128, EC, HD], F16, kind="ExternalInput")
    bdkv = nc.dram_tensor("bdkv", [128, LC], F32, kind="ExternalInput")
    bdq = nc.dram_tensor("bdq", [128, LC], F32, kind="ExternalInput")
    bkr = nc.dram_tensor("bkr", [128, 1], F32, kind="ExternalInput")
    bqr = nc.dram_tensor("bqr", [128, 1], F32, kind="ExternalInput")
    pet = nc.dram_tensor("pet", [HD, TPC], F32, kind="ExternalInput")
    wuk = nc.dram_tensor("wuk", [128, LC, HPC * HD], F16,
                         kind="ExternalInput")
    wuv = nc.dram_tensor("wuv", [128, LC, HPC * HD], F16,
                         kind="ExternalInput")
    wuq = nc.dram_tensor("wuq", [128, LC, HPC * HD], F16,
                         kind="ExternalInput")
    buk = nc.dram_tensor("buk", [128, HPC], F32, kind="ExternalInput")
    buv = nc.dram_tensor("buv", [128, HPC], F32, kind="ExternalInput")
    buq = nc.dram_tensor("buq", [128, HPC], F32, kind="ExternalInput")
    wo = nc.dram_tensor("wo", [128, HPC, DIM], F16, kind="ExternalInput")

    out_part = nc.dram_tensor("out_part", [TOK, DIM], F16,
                              kind="ExternalOutput")

    CKV_N = LAT * TPC            # 262144 elems per rank shard
    AUX_N = LAT * TPC + 2 * HD * TPC  # c_qT + k_rT + q_rT = 393216
    KR_OFF = LAT * TPC
    QR_OFF = KR_OFF + HD * TPC

    with tile.TileContext(nc) as tc:
        with tc.tile_pool(name="dram", bufs=1, space="DRAM") as dram:
            bin1 = dram.tile([1, CKV_N], F16)
            bout1 = dram.tile([RB, CKV_N], F16, addr_space="Shared")
            bin2 = dram.tile([1, AUX_N], F16)
            bout2 = dram.tile([RB, AUX_N], F16, addr_space="Shared")

            # ================= Phase A (token shard, transposed outputs) ===
            with tc.tile_pool(name="pA", bufs=1) as pA, \
                 tc.tile_pool(name="psA", bufs=3, space="PSUM") as psA:
                x_sb = pA.tile([128, EC, TPC], F16)
                nc.sync.dma_start(out=x_sb[:, 0:4], in_=xT[:, 0:4, :])
                nc.sync.dma_start(out=x_sb[:, 4:EC], in_=xT[:, 4:EC, :])
                wdkv_sb = pA.tile([128, EC, LAT], F16)
                nc.sync.dma_start(out=wdkv_sb[:, 0:4], in_=wdkv[:, 0:4, :])
                nc.sync.dma_start(out=wdkv_sb[:, 4:EC], in_=wdkv[:, 4:EC, :])
                wdq_sb = pA.tile([128, EC, LAT], F16)
                nc.sync.dma_start(out=wdq_sb, in_=wdq[:, :, :])
                wkr_sb = pA.tile([128, EC, HD], F16)
                nc.sync.dma_start(out=wkr_sb, in_=wkr[:, :, :])
                wqr_sb = pA.tile([128, EC, HD], F16)
                nc.sync.dma_start(out=wqr_sb, in_=wqr[:, :, :])
                bdkv_sb = pA.tile([128, LC], F32)
                nc.sync.dma_start(out=bdkv_sb, in_=bdkv[:, :])
                bdq_sb = pA.tile([128, LC], F32)
                nc.sync.dma_start(out=bdq_sb, in_=bdq[:, :])
                bkr_sb = pA.tile([128, 1], F32)
                nc.sync.dma_start(out=bkr_sb, in_=bkr[:, :])
                bqr_sb = pA.tile([128, 1], F32)
                nc.sync.dma_start(out=bqr_sb, in_=bqr[:, :])
                pet_sb = pA.tile([128, TPC], F32)
                nc.sync.dma_start(out=pet_sb, in_=pet[:, :])

                # c_kvT shard -> bounce1 -> AllGather #1 (early, overlaps rest)
                ckvT_sb = pA.tile([128, LC, TPC], F16)
                for lc in range(LC):
                    ps = psA.tile([128, TPC], F32, tag="ps_a")
                    for ec in range(EC):
                        nc.tensor.matmul(
                            ps[:, :],
                            wdkv_sb[:, ec, lc * 128:(lc + 1) * 128],
                            x_sb[:, ec, :],
                            start=(ec == 0), stop=(ec == EC - 1))
                    nc.scalar.add(ckvT_sb[:, lc, :], ps[:, :],
                                  bdkv_sb[:, lc:lc + 1])
                nc.sync.dma_start(
                    out=bin1[0, :].rearrange("(p n f) -> p n f", p=128, f=TPC),
                    in_=ckvT_sb)
                if use_cc:
                    nc.gpsimd.collective_compute(
                        "AllGather", mybir.AluOpType.bypass,
                        replica_groups=[list(range(N_CORES))],
                        ins=[bin1.opt()], outs=[bout1.opt()])

                # c_qT / ropes -> bounce2 -> AllGather #2
                cqT_sb = pA.tile([128, LC, TPC], F16)
                for lc in range(LC):
                    ps = psA.tile([128, TPC], F32, tag="ps_a")
                    for ec in range(EC):
                        nc.tensor.matmul(
                            ps[:, :],
                            wdq_sb[:, ec, lc * 128:(lc + 1) * 128],
                            x_sb[:, ec, :],
                            start=(ec == 0), stop=(ec == EC - 1))
                    nc.scalar.add(cqT_sb[:, lc, :], ps[:, :],
                                  bdq_sb[:, lc:lc + 1])
                krT_sb = pA.tile([128, TPC], F16)
                ps = psA.tile([128, TPC], F32, tag="ps_a")
                for ec in range(EC):
                    nc.tensor.matmul(ps[:, :], wkr_sb[:, ec, :], x_sb[:, ec, :],
                                     start=(ec == 0), stop=(ec == EC - 1))
                tmpr = pA.tile([128, TPC], F32)
                nc.scalar.add(tmpr[:, :], ps[:, :], bkr_sb[:, 0:1])
                nc.vector.tensor_mul(krT_sb[:, :], tmpr[:, :], pet_sb[:, :])
                qrT_sb = pA.tile([128, TPC], F16)
                ps = psA.tile([128, TPC], F32, tag="ps_a")
                for ec in range(EC):
                    nc.tensor.matmul(ps[:, :], wqr_sb[:, ec, :], x_sb[:, ec, :],
                                     start=(ec == 0), stop=(ec == EC - 1))
                tmpr2 = pA.tile([128, TPC], F32)
                nc.scalar.add(tmpr2[:, :], ps[:, :], bqr_sb[:, 0:1])
                nc.vector.tensor_mul(qrT_sb[:, :], tmpr2[:, :], pet_sb[:, :])

                nc.sync.dma_start(
                    out=bin2[0, 0:KR_OFF].rearrange(
                        "(p n f) -> p n f", p=128, f=TPC),
                    in_=cqT_sb)
                nc.sync.dma_start(
                    out=bin2[0, KR_OFF:QR_OFF].rearrange(
                        "(p f) -> p f", p=128), in_=krT_sb)
                nc.sync.dma_start(
                    out=bin2[0, QR_OFF:AUX_N].rearrange(
                        "(p f) -> p f", p=128), in_=qrT_sb)
                if use_cc:
                    nc.gpsimd.collective_compute(
                        "AllGather", mybir.AluOpType.bypass,
                        replica_groups=[list(range(N_CORES))],
                        ins=[bin2.opt()], outs=[bout2.opt()])

            # ================= Phase B: up-projections =====================
            _pB_cm = tc.tile_pool(name="pB", bufs=1)
            pB = _pB_cm.__enter__()
            ckv_sb = pB.tile([128, LC, RB, TPC], F16)
            for r in range(RB):
                nc.sync.dma_start(
                    out=ckv_sb[:, :, r, :],
                    in_=bout1[r, :].rearrange("(p n f) -> p n f",
                                              p=128, f=TPC))
            kr_sb = pB.tile([128, RB, TPC], F16)
            nc.sync.dma_start(
                out=kr_sb,
                in_=bout2[:, KR_OFF:QR_OFF].rearrange(
                    "r (p f) -> p r f", p=128))
            qr_sb = pB.tile([128, RB, TPC], F16)
            nc.sync.dma_start(
                out=qr_sb,
                in_=bout2[:, QR_OFF:AUX_N].rearrange(
                    "r (p f) -> p r f", p=128))
            kc_sb = pB.tile([128, HPC, RB, TPC], F16)
            qc_sb = pB.tile([128, HPC, RB, TPC], F16)
            v_sb = pB.tile([128, HPC, TOK // 128, 132], F16)
            wo_sb = pB.tile([128, HPC, DIM], F16)
            nc.sync.dma_start(out=wo_sb, in_=wo[:, :, :])
            buv_sb = pB.tile([128, HPC], F32)
            nc.sync.dma_start(out=buv_sb, in_=buv[:, :])
            ident = pB.tile([128, 128], F16)
            make_identity(nc, ident)
            nc.vector.memset(v_sb[:, :, :, 128:129], 1.0)

            with tc.tile_pool(name="pQ", bufs=1) as pQ, \
                 tc.tile_pool(name="psU", bufs=3, space="PSUM") as psU:
                cq_sb = pQ.tile([128, LC, RB, TPC], F16)
                for r in range(RB):
                    nc.sync.dma_start(
                        out=cq_sb[:, :, r, :],
                        in_=bout2[r, 0:KR_OFF].rearrange(
                            "(p n f) -> p n f", p=128, f=TPC))
                wuk_sb = pQ.tile([128, LC, HPC * HD], F16)
                nc.sync.dma_start(out=wuk_sb, in_=wuk[:, :, :])
                wuv_sb = pQ.tile([128, LC, HPC * HD], F16)
                nc.sync.dma_start(out=wuv_sb, in_=wuv[:, :, :])
                wuq_sb = pQ.tile([128, LC, HPC * HD], F16)
                nc.sync.dma_start(out=wuq_sb, in_=wuq[:, :, :])
                buk_sb = pQ.tile([128, HPC], F32)
                nc.sync.dma_start(out=buk_sb, in_=buk[:, :])
                buq_sb = pQ.tile([128, HPC], F32)
                nc.sync.dma_start(out=buq_sb, in_=buq[:, :])

                for h in range(HPC):
                    hs = slice(h * HD, (h + 1) * HD)
                    for rb in range(RB):
                        psk = psU.tile([128, TPC], F32, tag="ps_u")
                        for lc in range(LC):
                            nc.tensor.matmul(
                                psk[:, :], wuk_sb[:, lc, hs],
                                ckv_sb[:, lc, rb, :],
                                start=(lc == 0), stop=(lc == LC - 1))
                        nc.scalar.add(kc_sb[:, h, rb, :], psk[:, :],
                                      buk_sb[:, h:h + 1])
                        psq = psU.tile([128, TPC], F32, tag="ps_u")
                        for lc in range(LC):
                            nc.tensor.matmul(
                                psq[:, :], wuq_sb[:, lc, hs],
                                cq_sb[:, lc, rb, :],
                                start=(lc == 0), stop=(lc == LC - 1))
                        nc.scalar.add(qc_sb[:, h, rb, :], psq[:, :],
                                      buq_sb[:, h:h + 1])
                    for t in range(TOK // 128):
                        psv = psU.tile([128, 128], F32, tag="ps_v")
                        for lc in range(LC):
                            nc.tensor.matmul(
                                psv[:, :],
                                ckv_sb[:, lc, t // 4,
                                       (t % 4) * 128:(t % 4 + 1) * 128],
                                wuv_sb[:, lc, hs],
                                start=(lc == 0), stop=(lc == LC - 1))
                        nc.vector.tensor_copy(v_sb[:, h, t, 0:128], psv[:, :])

            # ================= Phase B: attention + output proj ============
            with tc.tile_pool(name="pAt", bufs=2) as pAt, \
                 tc.tile_pool(name="psS", bufs=o["psS_bufs"],
                              space="PSUM") as psS, \
                 tc.tile_pool(name="psC", bufs=o["psC_bufs"],
                              space="PSUM") as psC, \
                 tc.tile_pool(name="psT", bufs=o["psT_bufs"],
                              space="PSUM") as psT, \
                 tc.tile_pool(name="psO", bufs=o["psO_bufs"],
                              space="PSUM") as psO:
                for b in range(B):
                    if o["skip_attn"]:
                        break
                    for sb4 in range(NSB):
                        rq = 4 * b + sb4     # query rank-block
                        ctxT_sb = pAt.tile([128, HPC, SBLK // 128, 128], F16,
                                           tag="ctxT")
                        for h in range(HPC):
                            est_sb = pAt.tile([128, TC_B, SBLK], F16,
                                              tag="est", bufs=o["est_bufs"])
                            wexp = 2 if o["wide_exp"] else 1
                            for tp in range(TC_B // wexp):
                                # wide: two key chunks share one PSUM tile so
                                # the exp runs over 1024 elems per ACT instr
                                ps_s = psS.tile([128, wexp, SBLK], F32,
                                                tag="ps_s")
                                for ti in range(wexp):
                                    t = wexp * tp + ti
                                    rk = 4 * b + t // 4
                                    ko = (t % 4) * 128
                                    nc.tensor.matmul(
                                        ps_s[:, ti, :],
                                        kc_sb[:, h, rk, ko:ko + 128],
                                        qc_sb[:, h, rq, :],
                                        start=True, stop=False)
                                    nc.tensor.matmul(
                                        ps_s[:, ti, :],
                                        kr_sb[:, rk, ko:ko + 128],
                                        qr_sb[:, rq, :],
                                        start=False, stop=True)
                                nc.scalar.activation(
                                    est_sb[:, wexp * tp:wexp * (tp + 1), :],
                                    ps_s[:, :, :],
                                    mybir.ActivationFunctionType.Exp,
                                    scale=SCALE)
                            for sp in range(SBLK // 256):
                                if o["skip_ctx"]:
                                    break
                                ps_c = psC.tile([128, 2, 132], F32,
                                                tag="ps_c")
                                for si in range(2):
                                    ss = 2 * sp + si
                                    for t in range(TC_B):
                                        nc.tensor.matmul(
                                            ps_c[:, si, 0:129],
                                            est_sb[:, t,
                                                   ss * 128:(ss + 1) * 128],
                                            v_sb[:, h, TC_B * b + t, 0:129],
                                            start=(t == 0),
                                            stop=(t == TC_B - 1))
                                    recip = pAt.tile([128, 1], F32,
                                                     tag="recip")
                                    nc.vector.reciprocal(recip,
                                                         ps_c[:, si, 128:129])
                                    ctxn_sb = pAt.tile([128, 128], F16,
                                                       tag="ctxn")
                                    nc.vector.tensor_scalar_mul(
                                        ctxn_sb[:, :], ps_c[:, si, 0:128],
                                        recip)
                                    ps_t = psT.tile([128, 128], F16,
                                                    tag="ps_t")
                                    nc.tensor.transpose(ps_t[:, :],
                                                        ctxn_sb[:, :],
                                                        ident[:, :])
                                    nc.vector.tensor_scalar_add(
                                        ctxT_sb[:, h, ss, :], ps_t[:, :],
                                        buv_sb[:, h:h + 1])
                        n0 = (rq * TPC) // 128
                        out_dram = out_part.ap().rearrange(
                            "(n p) f -> p n f", p=128)
                        for ss in range(SBLK // 128):
                            if o["skip_ctx"]:
                                break
                            out_sb = pAt.tile([128, DIM], F16, tag="out")
                            for dt4 in range(DIM // 512):
                                ps_o = psO.tile([128, 512], F32, tag="ps_o")
                                for h in range(HPC):
                                    nc.tensor.matmul(
                                        ps_o[:, :],
                                        ctxT_sb[:, h, ss, :],
                                        wo_sb[:, h, dt4 * 512:(dt4 + 1) * 512],
                                        start=(h == 0), stop=(h == HPC - 1))
                                nc.vector.tensor_copy(
                                    out_sb[:, dt4 * 512:(dt4 + 1) * 512],
                                    ps_o[:, :])
                            nc.sync.dma_start(
                                out=out_dram[:, n0 + ss, :], in_=out_sb)
            _pB_cm.__exit__(None, None, None)

    nc.compile()
    return nc


def _rope_pe():
    pos = np.arange(S, dtype=np.float32)[:, None]
    div = np.exp(np.arange(0, HD, 2, dtype=np.float32)
                 * (-math.log(10000.0) / HD))
    pe = np.zeros((S, HD), dtype=np.float32)
    pe[:, 0::2] = np.sin(pos * div)
    pe[:, 1::2] = np.cos(pos * div)
    return pe


def _sbl(w, f16=True):
    """[n*128, C...] -> SBUF layout [128, n, C...] (partition-major)."""
    w = np.asarray(w, np.float32)
    n = w.shape[0] // 128
    out = np.ascontiguousarray(
        w.reshape(n, 128, *w.shape[1:]).swapaxes(0, 1))
    return out.astype(np.float16) if f16 else out


def _sblb(b):
    """bias [n*128] -> [128, n] fp32."""
    b = np.asarray(b, np.float32)
    n = b.size // 128
    return np.ascontiguousarray(b.reshape(n, 128).T)


def _prep_in_maps(inputs):
    f16 = np.float16
    x = np.asarray(inputs["x"], np.float32).reshape(TOK, DIM)
    pe = _rope_pe()
    shared = dict(
        wdkv=_sbl(inputs["W_DKV"]),
        wdq=_sbl(inputs["W_DQ"]),
        wkr=_sbl(inputs["W_KR"]),
        wqr=_sbl(inputs["W_QR"]),
        bdkv=_sblb(inputs["b_DKV"]),
        bdq=_sblb(inputs["b_DQ"]),
        bkr=_sblb(inputs["b_KR"]),
        bqr=_sblb(inputs["b_QR"]),
    )
    in_maps = []
    for r in range(N_CORES):
        tok = slice(r * TPC, (r + 1) * TPC)
        hslice = slice(r * HPC * HD, (r + 1) * HPC * HD)
        pos0 = (r * TPC) % S
        m = dict(shared)
        # xT sbuf layout: [128, EC, TPC]; x_sb[p, n, f] = x[tok_f, n*128+p]
        m["xT"] = np.ascontiguousarray(
            x[tok].T.reshape(EC, 128, TPC).swapaxes(0, 1)).astype(f16)
        m["pet"] = np.ascontiguousarray(pe[pos0:pos0 + TPC].T)
        m["wuk"] = _sbl(np.asarray(inputs["W_UK"], np.float32)[:, hslice])
        m["wuv"] = _sbl(np.asarray(inputs["W_UV"], np.float32)[:, hslice])
        m["wuq"] = _sbl(np.asarray(inputs["W_UQ"], np.float32)[:, hslice])
        m["buk"] = _sblb(np.asarray(inputs["b_UK"], np.float32)[hslice])
        m["buv"] = _sblb(np.asarray(inputs["b_UV"], np.float32)[hslice])
        m["buq"] = _sblb(np.asarray(inputs["b_UQ"], np.float32)[hslice])
        m["wo"] = _sbl(np.asarray(inputs["W_O"], np.float32)[hslice, :])
        in_maps.append(m)
    return in_maps


def _build_single(**opts):
    """Single-core, collective-free variant for cost-model timing."""
    return _build(use_cc=False, n_devices=1, **opts)


def _get_exec():
    """Build (once) a jitted shard_map executor over the 8 cores, mirroring
    concourse.bass2jax.run_bass_via_pjrt but cached so repeated kernel()
    calls do not re-trace/re-compile."""
    if "exec" in _CACHE:
        return _CACHE["exec"]
    import jax
    from jax.sharding import Mesh, PartitionSpec, NamedSharding
    from jax.experimental.shard_map import shard_map
    from concourse import bass2jax

    bass2jax.install_neuronx_cc_hook()
    if "nc" not in _CACHE:
        _CACHE["nc"] = _build()
    nc = _CACHE["nc"]

    _pname = nc.partition_id_tensor.name if nc.partition_id_tensor else None
    in_names, out_names, out_avals, zero_outs = [], [], [], []
    for alloc in nc.m.functions[0].allocations:
        if not isinstance(alloc, mybir.MemoryLocationSet):
            continue
        name = alloc.memorylocations[0].name
        if alloc.kind == "ExternalInput":
            if name != _pname:
                in_names.append(name)
        elif alloc.kind == "ExternalOutput":
            out_names.append(name)
            shape = tuple(alloc.tensor_shape)
            dtype = mybir.dt.np(alloc.dtype)
            out_avals.append(jax.core.ShapedArray(shape, dtype))
            zero_outs.append(np.zeros((N_CORES * shape[0], *shape[1:]), dtype))
    n_params = len(in_names)
    partition_name = (nc.partition_id_tensor.name
                      if nc.partition_id_tensor else None)
    all_names = in_names + out_names
    if partition_name is not None:
        all_names = all_names + [partition_name]

    def _body(*args):
        operands = list(args)
        if partition_name is not None:
            operands.append(bass2jax.partition_id_tensor())
        outs = bass2jax._bass_exec_p.bind(
            *operands,
            out_avals=tuple(out_avals),
            in_names=tuple(all_names),
            out_names=tuple(out_names),
            lowering_input_output_aliases=(),
            sim_require_finite=True,
            sim_require_nnan=True,
            nc=nc,
        )
        return tuple(outs)

    devices = jax.devices()[:N_CORES]
    mesh = Mesh(np.asarray(devices), ("core",))
    spec = PartitionSpec("core")
    in_specs = (spec,) * (n_params + len(out_names))
    out_specs = (spec,) * len(out_names)
    sharded = jax.jit(
        shard_map(_body, mesh=mesh, in_specs=in_specs, out_specs=out_specs,
                  check_rep=False),
        keep_unused=True,
    )
    sharding = NamedSharding(mesh, spec)
    zeros_dev = [jax.device_put(z, sharding) for z in zero_outs]
    _CACHE["exec"] = (sharded, in_names, out_names, out_avals, zeros_dev,
                      sharding)
    return _CACHE["exec"]


def _execute(in_maps):
    import jax
    sharded, in_names, out_names, out_avals, zeros_dev, sharding = _get_exec()
    concat_in = [
        np.concatenate([np.asarray(in_maps[c][n]) for c in range(N_CORES)],
                       axis=0)
        for n in in_names
    ]
    dev_in = [jax.device_put(a, sharding) for a in concat_in]
    out_arrs = sharded(*dev_in, *zeros_dev)
    out_arrs = [np.asarray(o) for o in out_arrs]
    return [
        {n: out_arrs[i].reshape(N_CORES, *out_avals[i].shape)[c]
         for i, n in enumerate(out_names)}
        for c in range(N_CORES)
    ]


def run(**inputs):
    in_maps = _prep_in_maps(inputs)
    results = _execute(in_maps)
    acc = np.zeros((TOK, DIM), np.float32)
    for r in range(N_CORES):
        acc += results[r]["out_part"].astype(np.float32)
    acc += np.asarray(inputs["b_O"], np.float32)
    return acc.reshape(B, S, DIM), results


def exec_only(in_maps):
    """For timing: run the prebuilt executor on preprocessed inputs."""
    return _execute(in_maps)


def timeit_loop(inputs, reps=20, n=6):
    """Chain `reps` kernel executions inside one jitted dispatch (the output
    zero-buffer is threaded as the carry, forcing sequential execution), so
    (T(reps) - T(1)) / (reps - 1) isolates per-execution device time from
    the tunnel/dispatch floor."""
    import time
    import jax
    from jax.sharding import Mesh, PartitionSpec, NamedSharding
    from jax.experimental.shard_map import shard_map
    from concourse import bass2jax

    bass2jax.install_neuronx_cc_hook()
    if "nc" not in _CACHE:
        _CACHE["nc"] = _build()
    nc = _CACHE["nc"]
    _pname = nc.partition_id_tensor.name if nc.partition_id_tensor else None
    in_names, out_names, out_avals, zero_outs = [], [], [], []
    for alloc in nc.m.functions[0].allocations:
        if not isinstance(alloc, mybir.MemoryLocationSet):
            continue
        name = alloc.memorylocations[0].name
        if alloc.kind == "ExternalInput":
            if name != _pname:
                in_names.append(name)
        elif alloc.kind == "ExternalOutput":
            out_names.append(name)
            shape = tuple(alloc.tensor_shape)
            dtype = mybir.dt.np(alloc.dtype)
            out_avals.append(jax.core.ShapedArray(shape, dtype))
            zero_outs.append(np.zeros((N_CORES * shape[0], *shape[1:]),
                                      dtype))
    all_names = in_names + out_names + ([_pname] if _pname else [])

    def _exec_once(args, carry):
        operands = list(args) + [carry]
        if _pname is not None:
            operands.append(bass2jax.partition_id_tensor())
        outs = bass2jax._bass_exec_p.bind(
            *operands, out_avals=tuple(out_avals),
            in_names=tuple(all_names), out_names=tuple(out_names),
            lowering_input_output_aliases=(),
            sim_require_finite=True, sim_require_nnan=True, nc=nc)
        return outs[0]

    def make_body(r):
        def _body(*args):
            carry = args[-1]
            ins = args[:-1]
            for _ in range(r):
                carry = _exec_once(ins, carry)
            return (carry,)
        return _body

    devices = jax.devices()[:N_CORES]
    mesh = Mesh(np.asarray(devices), ("core",))
    spec = PartitionSpec("core")
    sharding = NamedSharding(mesh, spec)
    in_maps = _prep_in_maps(inputs)
    dev_in = [
        jax.device_put(
            np.concatenate([np.asarray(in_maps[c][nm])
                            for c in range(N_CORES)], axis=0), sharding)
        for nm in in_names
    ]
    dev_z = jax.device_put(zero_outs[0], sharding)

    results = {}
    for r in (1, reps):
        f = jax.jit(
            shard_map(make_body(r), mesh=mesh,
                      in_specs=(spec,) * (len(in_names) + 1),
                      out_specs=(spec,), check_rep=False),
            keep_unused=True)
        outs = f(*dev_in, dev_z)
        jax.block_until_ready(outs)
        best = float("inf")
        for _ in range(n):
            t0 = time.perf_counter()
            outs = f(*dev_in, dev_z)
            jax.block_until_ready(outs)
            best = min(best, time.perf_counter() - t0)
        results[r] = best
    per_exec = (results[reps] - results[1]) / (reps - 1)
    return per_exec, results


def timeit(inputs, n=10):
    """Time the device execution with device-resident inputs (excludes
    host prep and H2D transfer; includes PJRT/tunnel dispatch)."""
    import time
    import jax
    in_maps = _prep_in_maps(inputs)
    sharded, in_names, _, _, zeros_dev, sharding = _get_exec()
    dev_in = [
        jax.device_put(
            np.concatenate([np.asarray(in_maps[c][nm])
                            for c in range(N_CORES)], axis=0), sharding)
        for nm in in_names
    ]
    outs = sharded(*dev_in, *zeros_dev)   # warm-up
    jax.block_until_ready(outs)
    times = []
    for _ in range(n):
        t0 = time.perf_counter()
        outs = sharded(*dev_in, *zeros_dev)
        jax.block_until_ready(outs)
        times.append(time.perf_counter() - t0)
    return times


def kernel(**inputs):
    out, _ = run(**inputs)
    return out


# revision 47
# speedup vs baseline: 120.9440x; 1.1237x over previous
"""Multi-Head Latent Attention (MLA) prefill kernel for 8 Trainium2 NeuronCores.

Problem shapes: B=2, S=2048, DIM=2048, H=16 heads, HEAD_DIM=128, LATENT=512.

Strategy (tensor-parallel over heads + data-parallel phase A):
  Phase A (token-DP): each core computes the latent down-projections
    c_kvT, c_qT and the rope projections k_rT, q_rT (pe-scaled) for its
    512-token shard, in transposed [feature, token] layout, fp16.
  AllGather (x2, overlapped): shards are exchanged so every core holds the
    full-sequence latents/ropes (~3.4MB/rank fp16 total).
  Phase B (head-TP): each core serves 2 of the 16 heads: up-projects
    k_c/q_c (transposed) and v (natural, with a ones-column appended so the
    softmax denominator falls out of the ctx matmul for free), then runs
    attention (scores^T tiles -> exp on ACT -> ctx accumulate in PSUM ->
    per-row normalize -> PE transpose -> W_O row-slice partial product).
  Host: sums the 8 partial outputs and adds b_O.

All matmuls run with fp16 operands and fp32 PSUM accumulation
(measured end-to-end rel. error ~3e-4 in numpy simulation).
"""
import math

import numpy as np

import concourse.bacc as bacc
import concourse.mybir as mybir
import concourse.tile as tile
from concourse.masks import make_identity

# Problem constants (hardcoded per harness contract).
B, S, DIM, H, HD, LAT = 2, 2048, 2048, 16, 128, 512
N_CORES = 8
HPC = H // N_CORES          # heads per core = 2
TOK = B * S                 # 4096 tokens
TPC = TOK // N_CORES        # 512 tokens per core (phase A shard)
EC = DIM // 128             # 16 embedding chunks
LC = LAT // 128             # 4 latent chunks
RB = N_CORES                # rank blocks of TPC tokens
SBLK = 512                  # query block (phase B)
NSB = S // SBLK             # 4 s-blocks per batch
TC_B = S // 128             # 16 key chunks per batch
F16 = mybir.dt.float16
F32 = mybir.dt.float32
SCALE = 1.0 / math.sqrt(HD)

_CACHE = {}


OPTS = dict(psS_bufs=2, psC_bufs=1, psT_bufs=1, psO_bufs=2, est_bufs=2,
            skip_attn=False, skip_ctx=False, wide_exp=True)


def _build(use_cc=True, n_devices=N_CORES, **opts):
    o = dict(OPTS)
    o.update(opts)
    nc = bacc.Bacc("TRN2", target_bir_lowering=False, debug=False,
                   num_devices=n_devices)

    # ---- per-core external inputs (host pre-permuted into SBUF layout so
    # every input DMA is a contiguous [128, X] copy) ----
    xT = nc.dram_tensor("xT", [128, EC, TPC], F16, kind="ExternalInput")
    wdkv = nc.dram_tensor("wdkv", [128, EC, LAT], F16, kind="ExternalInput")
    wdq = nc.dram_tensor("wdq", [128, EC, LAT], F16, kind="ExternalInput")
    wkr = nc.dram_tensor("wkr", [128, EC, HD], F16, kind="ExternalInput")
    wqr = nc.dram_tensor("wqr", [128, EC, HD], F16, kind="ExternalInput")
    bdkv = nc.dram_tensor("bdkv", [128, LC], F32, kind="ExternalInput")
    bdq = nc.dram_tensor("bdq", [128, LC], F32, kind="ExternalInput")
    bkr = nc.dram_tensor("bkr", [128, 1], F32, kind="ExternalInput")
    bqr = nc.dram_tensor("bqr", [128, 1], F32, kind="ExternalInput")
    pet = nc.dram_tensor("pet", [HD, TPC], F32, kind="ExternalInput")
    wuk = nc.dram_tensor("wuk", [128, LC, HPC * HD], F16,
                         kind="ExternalInput")
    wuv = nc.dram_tensor("wuv", [128, LC, HPC * HD], F16,
                         kind="ExternalInput")
    wuq = nc.dram_tensor("wuq", [128, LC, HPC * HD], F16,
                         kind="ExternalInput")
    buk = nc.dram_tensor("buk", [128, HPC], F32, kind="ExternalInput")
    buv = nc.dram_tensor("buv", [128, HPC], F32, kind="ExternalInput")
    buq = nc.dram_tensor("buq", [128, HPC], F32, kind="ExternalInput")
    wo = nc.dram_tensor("wo", [128, HPC, DIM], F16, kind="ExternalInput")

    out_part = nc.dram_tensor("out_part", [TOK, DIM], F16,
                              kind="ExternalOutput")

    CKV_N = LAT * TPC            # 262144 elems per rank shard
    AUX_N = LAT * TPC + 2 * HD * TPC  # c_qT + k_rT + q_rT = 393216
    KR_OFF = LAT * TPC
    QR_OFF = KR_OFF + HD * TPC

    with tile.TileContext(nc) as tc:
        with tc.tile_pool(name="dram", bufs=1, space="DRAM") as dram:
            bin1 = dram.tile([1, CKV_N], F16)
            bout1 = dram.tile([RB, CKV_N], F16, addr_space="Shared")
            bin2 = dram.tile([1, AUX_N], F16)
            bout2 = dram.tile([RB, AUX_N], F16, addr_space="Shared")

            # ================= Phase A (token shard, transposed outputs) ===
            with tc.tile_pool(name="pA", bufs=1) as pA, \
                 tc.tile_pool(name="psA", bufs=3, space="PSUM") as psA:
                x_sb = pA.tile([128, EC, TPC], F16)
                for _c in range(0, EC, 2):
                    nc.sync.dma_start(out=x_sb[:, _c:_c + 2],
                                      in_=xT[:, _c:_c + 2, :])
                wdkv_sb = pA.tile([128, EC, LAT], F16)
                for _c in range(0, EC, 2):
                    nc.sync.dma_start(out=wdkv_sb[:, _c:_c + 2],
                                      in_=wdkv[:, _c:_c + 2, :])
                wdq_sb = pA.tile([128, EC, LAT], F16)
                nc.sync.dma_start(out=wdq_sb, in_=wdq[:, :, :])
                wkr_sb = pA.tile([128, EC, HD], F16)
                nc.sync.dma_start(out=wkr_sb, in_=wkr[:, :, :])
                wqr_sb = pA.tile([128, EC, HD], F16)
                nc.sync.dma_start(out=wqr_sb, in_=wqr[:, :, :])
                bdkv_sb = pA.tile([128, LC], F32)
                nc.sync.dma_start(out=bdkv_sb, in_=bdkv[:, :])
                bdq_sb = pA.tile([128, LC], F32)
                nc.sync.dma_start(out=bdq_sb, in_=bdq[:, :])
                bkr_sb = pA.tile([128, 1], F32)
                nc.sync.dma_start(out=bkr_sb, in_=bkr[:, :])
                bqr_sb = pA.tile([128, 1], F32)
                nc.sync.dma_start(out=bqr_sb, in_=bqr[:, :])
                pet_sb = pA.tile([128, TPC], F32)
                nc.sync.dma_start(out=pet_sb, in_=pet[:, :])

                # c_kvT shard -> bounce1 -> AllGather #1 (early, overlaps rest)
                ckvT_sb = pA.tile([128, LC, TPC], F16)
                for lc in range(LC):
                    ps = psA.tile([128, TPC], F32, tag="ps_a")
                    for ec in range(EC):
                        nc.tensor.matmul(
                            ps[:, :],
                            wdkv_sb[:, ec, lc * 128:(lc + 1) * 128],
                            x_sb[:, ec, :],
                            start=(ec == 0), stop=(ec == EC - 1))
                    nc.scalar.add(ckvT_sb[:, lc, :], ps[:, :],
                                  bdkv_sb[:, lc:lc + 1])
                nc.sync.dma_start(
                    out=bin1[0, :].rearrange("(p n f) -> p n f", p=128, f=TPC),
                    in_=ckvT_sb)
                if use_cc:
                    nc.gpsimd.collective_compute(
                        "AllGather", mybir.AluOpType.bypass,
                        replica_groups=[list(range(N_CORES))],
                        ins=[bin1.opt()], outs=[bout1.opt()])

                # c_qT / ropes -> bounce2 -> AllGather #2
                cqT_sb = pA.tile([128, LC, TPC], F16)
                for lc in range(LC):
                    ps = psA.tile([128, TPC], F32, tag="ps_a")
                    for ec in range(EC):
                        nc.tensor.matmul(
                            ps[:, :],
                            wdq_sb[:, ec, lc * 128:(lc + 1) * 128],
                            x_sb[:, ec, :],
                            start=(ec == 0), stop=(ec == EC - 1))
                    nc.scalar.add(cqT_sb[:, lc, :], ps[:, :],
                                  bdq_sb[:, lc:lc + 1])
                krT_sb = pA.tile([128, TPC], F16)
                ps = psA.tile([128, TPC], F32, tag="ps_a")
                for ec in range(EC):
                    nc.tensor.matmul(ps[:, :], wkr_sb[:, ec, :], x_sb[:, ec, :],
                                     start=(ec == 0), stop=(ec == EC - 1))
                tmpr = pA.tile([128, TPC], F32)
                nc.scalar.add(tmpr[:, :], ps[:, :], bkr_sb[:, 0:1])
                nc.vector.tensor_mul(krT_sb[:, :], tmpr[:, :], pet_sb[:, :])
                qrT_sb = pA.tile([128, TPC], F16)
                ps = psA.tile([128, TPC], F32, tag="ps_a")
                for ec in range(EC):
                    nc.tensor.matmul(ps[:, :], wqr_sb[:, ec, :], x_sb[:, ec, :],
                                     start=(ec == 0), stop=(ec == EC - 1))
                tmpr2 = pA.tile([128, TPC], F32)
                nc.scalar.add(tmpr2[:, :], ps[:, :], bqr_sb[:, 0:1])
                nc.vector.tensor_mul(qrT_sb[:, :], tmpr2[:, :], pet_sb[:, :])

                nc.sync.dma_start(
                    out=bin2[0, 0:KR_OFF].rearrange(
                        "(p n f) -> p n f", p=128, f=TPC),
                    in_=cqT_sb)
                nc.sync.dma_start(
                    out=bin2[0, KR_OFF:QR_OFF].rearrange(
                        "(p f) -> p f", p=128), in_=krT_sb)
                nc.sync.dma_start(
                    out=bin2[0, QR_OFF:AUX_N].rearrange(
                        "(p f) -> p f", p=128), in_=qrT_sb)
                if use_cc:
                    nc.gpsimd.collective_compute(
                        "AllGather", mybir.AluOpType.bypass,
                        replica_groups=[list(range(N_CORES))],
                        ins=[bin2.opt()], outs=[bout2.opt()])

            # ================= Phase B: up-projections =====================
            _pB_cm = tc.tile_pool(name="pB", bufs=1)
            pB = _pB_cm.__enter__()
            ckv_sb = pB.tile([128, LC, RB, TPC], F16)
            for r in range(RB):
                nc.sync.dma_start(
                    out=ckv_sb[:, :, r, :],
                    in_=bout1[r, :].rearrange("(p n f) -> p n f",
                                              p=128, f=TPC))
            kr_sb = pB.tile([128, RB, TPC], F16)
            nc.sync.dma_start(
                out=kr_sb,
                in_=bout2[:, KR_OFF:QR_OFF].rearrange(
                    "r (p f) -> p r f", p=128))
            qr_sb = pB.tile([128, RB, TPC], F16)
            nc.sync.dma_start(
                out=qr_sb,
                in_=bout2[:, QR_OFF:AUX_N].rearrange(
                    "r (p f) -> p r f", p=128))
            kc_sb = pB.tile([128, HPC, RB, TPC], F16)
            qc_sb = pB.tile([128, HPC, RB, TPC], F16)
            v_sb = pB.tile([128, HPC, TOK // 128, 132], F16)
            wo_sb = pB.tile([128, HPC, DIM], F16)
            nc.sync.dma_start(out=wo_sb, in_=wo[:, :, :])
            buv_sb = pB.tile([128, HPC], F32)
            nc.sync.dma_start(out=buv_sb, in_=buv[:, :])
            ident = pB.tile([128, 128], F16)
            make_identity(nc, ident)
            nc.vector.memset(v_sb[:, :, :, 128:129], 1.0)

            with tc.tile_pool(name="pQ", bufs=1) as pQ, \
                 tc.tile_pool(name="psU", bufs=3, space="PSUM") as psU:
                cq_sb = pQ.tile([128, LC, RB, TPC], F16)
                for r in range(RB):
                    nc.sync.dma_start(
                        out=cq_sb[:, :, r, :],
                        in_=bout2[r, 0:KR_OFF].rearrange(
                            "(p n f) -> p n f", p=128, f=TPC))
                wuk_sb = pQ.tile([128, LC, HPC * HD], F16)
                nc.sync.dma_start(out=wuk_sb, in_=wuk[:, :, :])
                wuv_sb = pQ.tile([128, LC, HPC * HD], F16)
                nc.sync.dma_start(out=wuv_sb, in_=wuv[:, :, :])
                wuq_sb = pQ.tile([128, LC, HPC * HD], F16)
                nc.sync.dma_start(out=wuq_sb, in_=wuq[:, :, :])
                buk_sb = pQ.tile([128, HPC], F32)
                nc.sync.dma_start(out=buk_sb, in_=buk[:, :])
                buq_sb = pQ.tile([128, HPC], F32)
                nc.sync.dma_start(out=buq_sb, in_=buq[:, :])

                for h in range(HPC):
                    hs = slice(h * HD, (h + 1) * HD)
                    for rb in range(RB):
                        psk = psU.tile([128, TPC], F32, tag="ps_u")
                        for lc in range(LC):
                            nc.tensor.matmul(
                                psk[:, :], wuk_sb[:, lc, hs],
                                ckv_sb[:, lc, rb, :],
                                start=(lc == 0), stop=(lc == LC - 1))
                        nc.scalar.add(kc_sb[:, h, rb, :], psk[:, :],
                                      buk_sb[:, h:h + 1])
                        psq = psU.tile([128, TPC], F32, tag="ps_u")
                        for lc in range(LC):
                            nc.tensor.matmul(
                                psq[:, :], wuq_sb[:, lc, hs],
                                cq_sb[:, lc, rb, :],
                                start=(lc == 0), stop=(lc == LC - 1))
                        nc.scalar.add(qc_sb[:, h, rb, :], psq[:, :],
                                      buq_sb[:, h:h + 1])
                    for t in range(TOK // 128):
                        psv = psU.tile([128, 128], F32, tag="ps_v")
                        for lc in range(LC):
                            nc.tensor.matmul(
                                psv[:, :],
                                ckv_sb[:, lc, t // 4,
                                       (t % 4) * 128:(t % 4 + 1) * 128],
                                wuv_sb[:, lc, hs],
                                start=(lc == 0), stop=(lc == LC - 1))
                        nc.vector.tensor_copy(v_sb[:, h, t, 0:128], psv[:, :])

            # ================= Phase B: attention + output proj ============
            with tc.tile_pool(name="pAt", bufs=2) as pAt, \
                 tc.tile_pool(name="psS", bufs=o["psS_bufs"],
                              space="PSUM") as psS, \
                 tc.tile_pool(name="psC", bufs=o["psC_bufs"],
                              space="PSUM") as psC, \
                 tc.tile_pool(name="psT", bufs=o["psT_bufs"],
                              space="PSUM") as psT, \
                 tc.tile_pool(name="psO", bufs=o["psO_bufs"],
                              space="PSUM") as psO:
                for b in range(B):
                    if o["skip_attn"]:
                        break
                    for sb4 in range(NSB):
                        rq = 4 * b + sb4     # query rank-block
                        ctxT_sb = pAt.tile([128, HPC, SBLK // 128, 128], F16,
                                           tag="ctxT")
                        for h in range(HPC):
                            est_sb = pAt.tile([128, TC_B, SBLK], F16,
                                              tag="est", bufs=o["est_bufs"])
                            wexp = 2 if o["wide_exp"] else 1
                            for tp in range(TC_B // wexp):
                                # wide: two key chunks share one PSUM tile so
                                # the exp runs over 1024 elems per ACT instr
                                ps_s = psS.tile([128, wexp, SBLK], F32,
                                                tag="ps_s")
                                for ti in range(wexp):
                                    t = wexp * tp + ti
                                    rk = 4 * b + t // 4
                                    ko = (t % 4) * 128
                                    nc.tensor.matmul(
                                        ps_s[:, ti, :],
                                        kc_sb[:, h, rk, ko:ko + 128],
                                        qc_sb[:, h, rq, :],
                                        start=True, stop=False)
                                    nc.tensor.matmul(
                                        ps_s[:, ti, :],
                                        kr_sb[:, rk, ko:ko + 128],
                                        qr_sb[:, rq, :],
                                        start=False, stop=True)
                                nc.scalar.activation(
                                    est_sb[:, wexp * tp:wexp * (tp + 1), :],
                                    ps_s[:, :, :],
                                    mybir.ActivationFunctionType.Exp,
                                    scale=SCALE)
                            for sp in range(SBLK // 256):
                                if o["skip_ctx"]:
                                    break
                                ps_c = psC.tile([128, 2, 132], F32,
                                                tag="ps_c")
                                for si in range(2):
                                    ss = 2 * sp + si
                                    for t in range(TC_B):
                                        nc.tensor.matmul(
                                            ps_c[:, si, 0:129],
                                            est_sb[:, t,
                                                   ss * 128:(ss + 1) * 128],
                                            v_sb[:, h, TC_B * b + t, 0:129],
                                            start=(t == 0),
                                            stop=(t == TC_B - 1))
                                    recip = pAt.tile([128, 1], F32,
                                                     tag="recip")
                                    nc.vector.reciprocal(recip,
                                                         ps_c[:, si, 128:129])
                                    ctxn_sb = pAt.tile([128, 128], F16,
                                                       tag="ctxn")
                                    nc.vector.tensor_scalar_mul(
                                        ctxn_sb[:, :], ps_c[:, si, 0:128],
                                        recip)
                                    ps_t = psT.tile([128, 128], F16,
                                                    tag="ps_t")
                                    nc.tensor.transpose(ps_t[:, :],
                                                        ctxn_sb[:, :],
                                                        ident[:, :])
                                    nc.vector.tensor_scalar_add(
                                        ctxT_sb[:, h, ss, :], ps_t[:, :],
                                        buv_sb[:, h:h + 1])
                        n0 = (rq * TPC) // 128
                        out_dram = out_part.ap().rearrange(
                            "(n p) f -> p n f", p=128)
                        for ss in range(SBLK // 128):
                            if o["skip_ctx"]:
                                break
                            out_sb = pAt.tile([128, DIM], F16, tag="out")
                            for dt4 in range(DIM // 512):
                                ps_o = psO.tile([128, 512], F32, tag="ps_o")
                                for h in range(HPC):
                                    nc.tensor.matmul(
                                        ps_o[:, :],
                                        ctxT_sb[:, h, ss, :],
                                        wo_sb[:, h, dt4 * 512:(dt4 + 1) * 512],
                                        start=(h == 0), stop=(h == HPC - 1))
                                nc.vector.tensor_copy(
                                    out_sb[:, dt4 * 512:(dt4 + 1) * 512],
                                    ps_o[:, :])
                            nc.sync.dma_start(
                                out=out_dram[:, n0 + ss, :], in_=out_sb)
            _pB_cm.__exit__(None, None, None)

    nc.compile()
    return nc


def _rope_pe():
    pos = np.arange(S, dtype=np.float32)[:, None]
    div = np.exp(np.arange(0, HD, 2, dtype=np.float32)
                 * (-math.log(10000.0) / HD))
    pe = np.zeros((S, HD), dtype=np.float32)
    pe[:, 0::2] = np.sin(pos * div)
    pe[:, 1::2] = np.cos(pos * div)
    return pe


def _sbl(w, f16=True):
    """[n*128, C...] -> SBUF layout [128, n, C...] (partition-major)."""
    w = np.asarray(w, np.float32)
    n = w.shape[0] // 128
    out = np.ascontiguousarray(
        w.reshape(n, 128, *w.shape[1:]).swapaxes(0, 1))
    return out.astype(np.float16) if f16 else out


def _sblb(b):
    """bias [n*128] -> [128, n] fp32."""
    b = np.asarray(b, np.float32)
    n = b.size // 128
    return np.ascontiguousarray(b.reshape(n, 128).T)


def _prep_in_maps(inputs):
    f16 = np.float16
    x = np.asarray(inputs["x"], np.float32).reshape(TOK, DIM)
    pe = _rope_pe()
    shared = dict(
        wdkv=_sbl(inputs["W_DKV"]),
        wdq=_sbl(inputs["W_DQ"]),
        wkr=_sbl(inputs["W_KR"]),
        wqr=_sbl(inputs["W_QR"]),
        bdkv=_sblb(inputs["b_DKV"]),
        bdq=_sblb(inputs["b_DQ"]),
        bkr=_sblb(inputs["b_KR"]),
        bqr=_sblb(inputs["b_QR"]),
    )
    in_maps = []
    for r in range(N_CORES):
        tok = slice(r * TPC, (r + 1) * TPC)
        hslice = slice(r * HPC * HD, (r + 1) * HPC * HD)
        pos0 = (r * TPC) % S
        m = dict(shared)
        # xT sbuf layout: [128, EC, TPC]; x_sb[p, n, f] = x[tok_f, n*128+p]
        m["xT"] = np.ascontiguousarray(
            x[tok].T.reshape(EC, 128, TPC).swapaxes(0, 1)).astype(f16)
        m["pet"] = np.ascontiguousarray(pe[pos0:pos0 + TPC].T)
        m["wuk"] = _sbl(np.asarray(inputs["W_UK"], np.float32)[:, hslice])
        m["wuv"] = _sbl(np.asarray(inputs["W_UV"], np.float32)[:, hslice])
        m["wuq"] = _sbl(np.asarray(inputs["W_UQ"], np.float32)[:, hslice])
        m["buk"] = _sblb(np.asarray(inputs["b_UK"], np.float32)[hslice])
        m["buv"] = _sblb(np.asarray(inputs["b_UV"], np.float32)[hslice])
        m["buq"] = _sblb(np.asarray(inputs["b_UQ"], np.float32)[hslice])
        m["wo"] = _sbl(np.asarray(inputs["W_O"], np.float32)[hslice, :])
        in_maps.append(m)
    return in_maps


def _build_single(**opts):
    """Single-core, collective-free variant for cost-model timing."""
    return _build(use_cc=False, n_devices=1, **opts)


def _get_exec():
    """Build (once) a jitted shard_map executor over the 8 cores, mirroring
    concourse.bass2jax.run_bass_via_pjrt but cached so repeated kernel()
    calls do not re-trace/re-compile."""
    if "exec" in _CACHE:
        return _CACHE["exec"]
    import jax
    from jax.sharding import Mesh, PartitionSpec, NamedSharding
    from jax.experimental.shard_map import shard_map
    from concourse import bass2jax

    bass2jax.install_neuronx_cc_hook()
    if "nc" not in _CACHE:
        _CACHE["nc"] = _build()
    nc = _CACHE["nc"]

    _pname = nc.partition_id_tensor.name if nc.partition_id_tensor else None
    in_names, out_names, out_avals, zero_outs = [], [], [], []
    for alloc in nc.m.functions[0].allocations:
        if not isinstance(alloc, mybir.MemoryLocationSet):
            continue
        name = alloc.memorylocations[0].name
        if alloc.kind == "ExternalInput":
            if name != _pname:
                in_names.append(name)
        elif alloc.kind == "ExternalOutput":
            out_names.append(name)
            shape = tuple(alloc.tensor_shape)
            dtype = mybir.dt.np(alloc.dtype)
            out_avals.append(jax.core.ShapedArray(shape, dtype))
            zero_outs.append(np.zeros((N_CORES * shape[0], *shape[1:]), dtype))
    n_params = len(in_names)
    partition_name = (nc.partition_id_tensor.name
                      if nc.partition_id_tensor else None)
    all_names = in_names + out_names
    if partition_name is not None:
        all_names = all_names + [partition_name]

    def _body(*args):
        operands = list(args)
        if partition_name is not None:
            operands.append(bass2jax.partition_id_tensor())
        outs = bass2jax._bass_exec_p.bind(
            *operands,
            out_avals=tuple(out_avals),
            in_names=tuple(all_names),
            out_names=tuple(out_names),
            lowering_input_output_aliases=(),
            sim_require_finite=True,
            sim_require_nnan=True,
            nc=nc,
        )
        return tuple(outs)

    devices = jax.devices()[:N_CORES]
    mesh = Mesh(np.asarray(devices), ("core",))
    spec = PartitionSpec("core")
    in_specs = (spec,) * (n_params + len(out_names))
    out_specs = (spec,) * len(out_names)
    sharded = jax.jit(
        shard_map(_body, mesh=mesh, in_specs=in_specs, out_specs=out_specs,
                  check_rep=False),
        keep_unused=True,
    )
    sharding = NamedSharding(mesh, spec)
    zeros_dev = [jax.device_put(z, sharding) for z in zero_outs]
    _CACHE["exec"] = (sharded, in_names, out_names, out_avals, zeros_dev,
                      sharding)
    return _CACHE["exec"]


def _execute(in_maps):
    import jax
    sharded, in_names, out_names, out_avals, zeros_dev, sharding = _get_exec()
    concat_in = [
        np.concatenate([np.asarray(in_maps[c][n]) for c in range(N_CORES)],
                       axis=0)
        for n in in_names
    ]
    dev_in = [jax.device_put(a, sharding) for a in concat_in]
    out_arrs = sharded(*dev_in, *zeros_dev)
    out_arrs = [np.asarray(o) for o in out_arrs]
    return [
        {n: out_arrs[i].reshape(N_CORES, *out_avals[i].shape)[c]
         for i, n in enumerate(out_names)}
        for c in range(N_CORES)
    ]


def run(**inputs):
    in_maps = _prep_in_maps(inputs)
    results = _execute(in_maps)
    acc = np.zeros((TOK, DIM), np.float32)
    for r in range(N_CORES):
        acc += results[r]["out_part"].astype(np.float32)
    acc += np.asarray(inputs["b_O"], np.float32)
    return acc.reshape(B, S, DIM), results


def exec_only(in_maps):
    """For timing: run the prebuilt executor on preprocessed inputs."""
    return _execute(in_maps)


def timeit_loop(inputs, reps=20, n=6):
    """Chain `reps` kernel executions inside one jitted dispatch (the output
    zero-buffer is threaded as the carry, forcing sequential execution), so
    (T(reps) - T(1)) / (reps - 1) isolates per-execution device time from
    the tunnel/dispatch floor."""
    import time
    import jax
    from jax.sharding import Mesh, PartitionSpec, NamedSharding
    from jax.experimental.shard_map import shard_map
    from concourse import bass2jax

    bass2jax.install_neuronx_cc_hook()
    if "nc" not in _CACHE:
        _CACHE["nc"] = _build()
    nc = _CACHE["nc"]
    _pname = nc.partition_id_tensor.name if nc.partition_id_tensor else None
    in_names, out_names, out_avals, zero_outs = [], [], [], []
    for alloc in nc.m.functions[0].allocations:
        if not isinstance(alloc, mybir.MemoryLocationSet):
            continue
        name = alloc.memorylocations[0].name
        if alloc.kind == "ExternalInput":
            if name != _pname:
                in_names.append(name)
        elif alloc.kind == "ExternalOutput":
            out_names.append(name)
            shape = tuple(alloc.tensor_shape)
            dtype = mybir.dt.np(alloc.dtype)
            out_avals.append(jax.core.ShapedArray(shape, dtype))
            zero_outs.append(np.zeros((N_CORES * shape[0], *shape[1:]),
                                      dtype))
    all_names = in_names + out_names + ([_pname] if _pname else [])

    def _exec_once(args, carry):
        operands = list(args) + [carry]
        if _pname is not None:
            operands.append(bass2jax.partition_id_tensor())
        outs = bass2jax._bass_exec_p.bind(
            *operands, out_avals=tuple(out_avals),
            in_names=tuple(all_names), out_names=tuple(out_names),
            lowering_input_output_aliases=(),
            sim_require_finite=True, sim_require_nnan=True, nc=nc)
        return outs[0]

    def make_body(r):
        def _body(*args):
            carry = args[-1]
            ins = args[:-1]
            for _ in range(r):
                carry = _exec_once(ins, carry)
            return (carry,)
        return _body

    devices = jax.devices()[:N_CORES]
    mesh = Mesh(np.asarray(devices), ("core",))
    spec = PartitionSpec("core")
    sharding = NamedSharding(mesh, spec)
    in_maps = _prep_in_maps(inputs)
    dev_in = [
        jax.device_put(
            np.concatenate([np.asarray(in_maps[c][nm])
                            for c in range(N_CORES)], axis=0), sharding)
        for nm in in_names
    ]
    dev_z = jax.device_put(zero_outs[0], sharding)

    results = {}
    for r in (1, reps):
        f = jax.jit(
            shard_map(make_body(r), mesh=mesh,
                      in_specs=(spec,) * (len(in_names) + 1),
                      out_specs=(spec,), check_rep=False),
            keep_unused=True)
        outs = f(*dev_in, dev_z)
        jax.block_until_ready(outs)
        best = float("inf")
        for _ in range(n):
            t0 = time.perf_counter()
            outs = f(*dev_in, dev_z)
            jax.block_until_ready(outs)
            best = min(best, time.perf_counter() - t0)
        results[r] = best
    per_exec = (results[reps] - results[1]) / (reps - 1)
    return per_exec, results


def timeit(inputs, n=10):
    """Time the device execution with device-resident inputs (excludes
    host prep and H2D transfer; includes PJRT/tunnel dispatch)."""
    import time
    import jax
    in_maps = _prep_in_maps(inputs)
    sharded, in_names, _, _, zeros_dev, sharding = _get_exec()
    dev_in = [
        jax.device_put(
            np.concatenate([np.asarray(in_maps[c][nm])
                            for c in range(N_CORES)], axis=0), sharding)
        for nm in in_names
    ]
    outs = sharded(*dev_in, *zeros_dev)   # warm-up
    jax.block_until_ready(outs)
    times = []
    for _ in range(n):
        t0 = time.perf_counter()
        outs = sharded(*dev_in, *zeros_dev)
        jax.block_until_ready(outs)
        times.append(time.perf_counter() - t0)
    return times


def kernel(**inputs):
    out, _ = run(**inputs)
    return out
